# revision 1
# baseline (speedup 1.0000x reference)
"""Trainium2 Bass kernel for nn_FuncSelfAttention (spectral self-attention).

Math: the spectral convs keep only 2x2 Fourier modes, so rfft2/irfft2 collapse
to a [1024 -> 8] projection (E8) and an [8 -> 1024] reconstruction (Bas).  The
whole network runs in the 8-dim mode/coefficient space; attention inner
products over (hd, H, W) reduce to a diagonal 8x8 Gram matrix.  The only large
data movement is reading seq (128 MiB) and writing the output (128 MiB) =>
memory-bound.  Sharding: data-parallel over batch (B=8) across 8 cores.

Per core: x [4096=(s,c), 1024=(h,w)] ->
  stage 1: PE-transpose x chunks, project with E8 -> mode coords XR/XI [c,(m,s)]
  phase 2: complex channel mixing (w_qkv) -> Q/K/V coeffs [s, (jm,h,d)]
  attn:    per head: scores = (g-scaled Uq)^T Uk + cpb bias, softmax, attn @ V
  phase 6: T8-diag scale + w_out mixing -> final coeffs U_fT [8, rows]
  stage 7: y = U_fT^T @ Bas -> [4096, 1024] streamed out.
"""
import numpy as np

B, S, C, H, W = 8, 64, 64, 32, 32
NH, HD = 8, 8
HW = H * W
NCORES = 8
MODES4 = [(0, 0), (0, 1), (1, 0), (1, 1)]

# DT_BIG: dtype for the two big matmul paths (stage 1 projection, stage 7
# reconstruction) and their transposed operands.  "f32r" = fp32-replicated
# (full-rate on PE), "bf16", or "f32" (exact, 4x slower PE).
DT_BIG = "f16"


def _constants():
    hh, ww = np.meshgrid(np.arange(H), np.arange(W), indexing="ij")
    phi, psi = 2 * np.pi / H, 2 * np.pi / W
    E8 = np.zeros((HW, 8))
    Bas = np.zeros((8, HW))
    for mi, (kx, ky) in enumerate(MODES4):
        th = phi * kx * hh + psi * ky * ww
        E8[:, 2 * mi] = np.cos(th).ravel()
        E8[:, 2 * mi + 1] = -np.sin(th).ravel()
        mult = 1.0 if ky == 0 else 2.0
        Bas[2 * mi] = mult / HW * np.cos(th).ravel()
        Bas[2 * mi + 1] = -mult / HW * np.sin(th).ravel()
    g = (Bas @ Bas.T).diagonal().copy()      # attention Gram diag
    t8d = (Bas @ E8).diagonal().copy()       # coeff->mode map (diagonal)

    e8c = np.zeros((128, 64), np.float32)    # chunk k at cols [8k, 8k+8)
    for k in range(8):
        e8c[:, 8 * k:8 * k + 8] = E8[128 * k:128 * (k + 1)]

    gx, gy = np.meshgrid(np.arange(8), np.arange(8), indexing="ij")
    coords = np.stack([gx.ravel(), gy.ravel()], -1).astype(np.float32)
    rel = coords[:, None, :] - coords[None, :, :]
    rel = np.sign(rel) * np.log2(1.0 + np.abs(rel))          # [64, 64, 2]
    relT = np.ascontiguousarray(rel.reshape(4096, 2).T).astype(np.float32)

    scale = np.float32(1.0 / HW) / np.float32(np.sqrt(HD))
    gcol = np.zeros((64, 1), np.float32)     # dj order = (jm, d): p = jm*8+d
    for p in range(64):
        gcol[p, 0] = g[p // 8] * scale
    t8pat = np.zeros((1, 512), np.float32)   # over (jm, h, d): col = jm*64+..
    for jm in range(8):
        t8pat[0, jm * 64:(jm + 1) * 64] = t8d[jm]
    return e8c, Bas.astype(np.float32), relT, gcol, t8pat


def _build(dt_big_name=DT_BIG):
    import concourse.bass as bass
    import concourse.mybir as mybir
    import concourse.tile as tile
    from concourse import bacc
    from concourse.masks import make_identity

    f32 = mybir.dt.float32
    dt_big = {"f32r": mybir.dt.float32r, "bf16": mybir.dt.bfloat16,
              "f16": mybir.dt.float16, "f32": f32}[dt_big_name]
    cast_on_load = dt_big_name in ("bf16", "f16")
    Exp = mybir.ActivationFunctionType.Exp
    Relu = mybir.ActivationFunctionType.Relu

    NXBAR = 0   # tiles per group transposed via DMA xbar (rest on PE)
    nc = bacc.Bacc("TRN2", target_bir_lowering=False, debug=False,
                   dynamic_dma_scratch_size=49152)
    x_in = nc.dram_tensor("x", [4096, 1024], f32, kind="ExternalInput")
    wqr_in = nc.dram_tensor("wqr", [64, 768], f32, kind="ExternalInput")
    wqi_in = nc.dram_tensor("wqi", [64, 768], f32, kind="ExternalInput")
    wor_in = nc.dram_tensor("wor", [64, 256], f32, kind="ExternalInput")
    woi_in = nc.dram_tensor("woi", [64, 256], f32, kind="ExternalInput")
    cw1_in = nc.dram_tensor("cw1", [2, 64], f32, kind="ExternalInput")
    cb1_in = nc.dram_tensor("cb1", [64, 1], f32, kind="ExternalInput")
    cw2_in = nc.dram_tensor("cw2", [64, 8], f32, kind="ExternalInput")
    e8_in = nc.dram_tensor("e8c", [128, 64], f32, kind="ExternalInput")
    bas_in = nc.dram_tensor("bas", [8, 1024], f32, kind="ExternalInput")
    rel_in = nc.dram_tensor("relT", [2, 4096], f32, kind="ExternalInput")
    gcol_in = nc.dram_tensor("gcol", [64, 1], f32, kind="ExternalInput")
    t8_in = nc.dram_tensor("t8pat", [1, 512], f32, kind="ExternalInput")
    y_out = nc.dram_tensor("y", [4096, 1024],
                           mybir.dt.float16, kind="ExternalOutput")

    with tile.TileContext(nc) as tc:
        import contextlib
        ctx = contextlib.ExitStack()
        with ctx:
            singles = ctx.enter_context(tc.tile_pool(name="singles", bufs=1))
            ps = ctx.enter_context(tc.tile_pool(name="ps", bufs=5, space="PSUM"))
            psl = ctx.enter_context(tc.tile_pool(name="psl", bufs=1, space="PSUM"))
            x_pool = ctx.enter_context(tc.tile_pool(name="xp", bufs=3))
            xt_pool = ctx.enter_context(tc.tile_pool(name="xt", bufs=5))
            m_pool = ctx.enter_context(tc.tile_pool(name="mp", bufs=3))
            sm_pool = ctx.enter_context(tc.tile_pool(name="sm", bufs=6))
            y_pool = ctx.enter_context(tc.tile_pool(name="yp", bufs=8))

            # ---- constants / weights into SBUF ----
            def load1(name, dram, shape):
                t = singles.tile(shape, f32, tag=name)
                nc.sync.dma_start(out=t[:], in_=dram[:])
                return t

            e8_f = load1("e8", e8_in, [128, 64])
            bas_f = load1("bas", bas_in, [8, 1024])
            relT = load1("relT", rel_in, [2, 4096])
            gcol = load1("gcol", gcol_in, [64, 1])
            wqr = load1("wqr", wqr_in, [64, 768])
            wqi = load1("wqi", wqi_in, [64, 768])
            wor = load1("wor", wor_in, [64, 256])
            woi = load1("woi", woi_in, [64, 256])
            cw1 = load1("cw1", cw1_in, [2, 64])
            cb1 = load1("cb1", cb1_in, [64, 1])
            cw2 = load1("cw2", cw2_in, [64, 8])
            t8rep = singles.tile([64, 512], f32, tag="t8rep")
            nc.sync.dma_start(out=t8rep[:], in_=t8_in[:].to_broadcast([64, 512]))

            ident = singles.tile([128, 128], f32, tag="ident")
            make_identity(nc, ident[:])

            dt_mid = dt_big if dt_big_name == "f16" else f32
            wqrm = singles.tile([64, 768], dt_mid, tag="wqrm")
            nc.vector.tensor_copy(wqrm[:], wqi[:]) if False else nc.vector.tensor_copy(wqrm[:], wqr[:])
            wqim = singles.tile([64, 768], dt_mid, tag="wqim")
            nc.vector.tensor_copy(wqim[:], wqi[:])
            wqin = singles.tile([64, 768], dt_mid, tag="wqin")
            nc.vector.tensor_scalar_mul(wqin[:], wqi[:], -1.0)
            worm = singles.tile([64, 256], dt_mid, tag="worm")
            nc.vector.tensor_copy(worm[:], wor[:])
            woim = singles.tile([64, 256], dt_mid, tag="woim")
            nc.vector.tensor_copy(woim[:], woi[:])
            woin = singles.tile([64, 256], dt_mid, tag="woin")
            nc.vector.tensor_scalar_mul(woin[:], woi[:], -1.0)

            # big-path operands in dt_big
            if dt_big != f32:
                e8b = singles.tile([128, 64], dt_big, tag="e8b")
                nc.vector.tensor_copy(e8b[:], e8_f[:])
                basb = singles.tile([8, 1024], dt_big, tag="basb")
                nc.vector.tensor_copy(basb[:], bas_f[:])
                identb = singles.tile([128, 128], dt_big, tag="identb")
                nc.vector.tensor_copy(identb[:], ident[:])
            else:
                e8b, basb, identb = e8_f, bas_f, ident

            # persistent intermediates
            XR = singles.tile([64, 256], dt_mid, tag="XR")   # [c, (m, s)]
            XI = singles.tile([64, 256], dt_mid, tag="XI")
            h_relu = singles.tile([64, 4096], dt_mid, tag="hrelu")
            bias_sb = singles.tile([64, 512], f32, tag="bias")   # [i, (h, j)]
            Q_sb = singles.tile([64, 512], f32, tag="Qsb")  # [s, (jm, h, d)]
            K_sb = singles.tile([64, 512], f32, tag="Ksb")
            V_sb = singles.tile([64, 512], f32, tag="Vsb")
            O_all = singles.tile([64, 512], f32, tag="Oall")  # [i, (jm, h, d)]
            O_sc = singles.tile([64, 512], f32, tag="Osc")
            XOR = singles.tile([64, 256], dt_mid, tag="XOR")  # [c, (m, s)]
            XOI = singles.tile([64, 256], dt_mid, tag="XOI")
            F_sb = singles.tile([64, 512], f32, tag="Fsb")   # [c_out, (jm, s)]
            U_fT = singles.tile([8, 4096], dt_big, tag="UfT")  # [jm, rows]

            # ---- stage 1: transpose + project; 8 groups of 512 rows ----
            xt_dt = dt_big if cast_on_load else f32
            tr_ident = identb if cast_on_load else ident
            for gi in range(8):
                xTg = xt_pool.tile([128, 4096], dt_big, tag="xTg")
                xTg4 = xTg.rearrange("p (k t r) -> p k t r", k=8, t=4)
                x_q = x_pool.tile([128, 4096], xt_dt, tag="x_q")
                for half in range(2):
                    srcv = x_in[512 * gi + 256 * half:512 * gi + 256 * (half + 1),
                                :].rearrange("(t p) q -> p t q", p=128)
                    nc.gpsimd.dma_start(
                        out=x_q[:, 2048 * half:2048 * (half + 1)], in_=srcv)
                xq3 = x_q.rearrange("p (t q) -> p t q", t=4)
                for t in range(4):
                    if t < NXBAR:
                        # hw xbar: out[col%128, col//128, row] = in[row, col]
                        # -> xTg4[:, :, t, :][qm, k, r] = x_t[r, 128k+qm]
                        eng = nc.sync if t % 2 == 0 else nc.scalar
                        eng.dma_start_transpose(out=xTg4[:, :, t, :],
                                                in_=xq3[:, t, :])
                        continue
                    x_t = xq3[:, t, :]
                    for a in range(2):
                        ptr = ps.tile([128, 512], xt_dt, tag="ps")
                        for j in range(4):
                            k = 4 * a + j
                            nc.tensor.transpose(ptr[:, 128 * j:128 * (j + 1)],
                                                x_t[:, 128 * k:128 * (k + 1)],
                                                tr_ident[:])
                        dst = xTg4[:, 4 * a:4 * a + 4, t, :]
                        if (2 * t + a) % 3 < 2:
                            nc.vector.tensor_copy(dst, ptr[:])
                        else:
                            nc.scalar.copy(dst, ptr[:])
                pm = ps.tile([8, 512], f32, tag="ps")
                for k in range(8):
                    nc.tensor.matmul(pm[:], e8b[:, 8 * k:8 * k + 8],
                                     xTg[:, 512 * k:512 * (k + 1)],
                                     start=(k == 0), stop=(k == 7))
                m_sb = m_pool.tile([8, 512], dt_mid, tag="m_sb")
                nc.vector.tensor_copy(m_sb[:], pm[:])
                # per-s transposes [8, 64] -> [64, 8], all into one psum [64, 64]
                pxg = ps.tile([64, 64], dt_mid, tag="ps")
                tid = identb if dt_mid != f32 else ident
                for u in range(8):
                    nc.tensor.transpose(pxg[:, 8 * u:8 * u + 8],
                                        m_sb[:, 64 * u:64 * (u + 1)], tid[:8, :8])
                # scatter to XR/XI: src (c, u, m, t) -> dst (c, m, s=8g+u)
                pxv = pxg.rearrange("c (u m t) -> c m u t", m=4, t=2)
                xr3 = XR.rearrange("c (m s) -> c m s", s=64)
                xi3 = XI.rearrange("c (m s) -> c m s", s=64)
                nc.vector.tensor_copy(xr3[:, :, 8 * gi:8 * gi + 8], pxv[:, :, :, 0])
                nc.vector.tensor_copy(xi3[:, :, 8 * gi:8 * gi + 8], pxv[:, :, :, 1])

            # ---- CPB bias: relu(relT^T @ cw1 + b1) @ cw2 -> [i, (h, j)] ----
            if dt_big != f32:
                relTb = singles.tile([2, 4096], dt_big, tag="relTb")
                nc.vector.tensor_copy(relTb[:], relT[:])
                cw1b = singles.tile([2, 64], dt_big, tag="cw1b")
                nc.vector.tensor_copy(cw1b[:], cw1[:])
            else:
                relTb, cw1b = relT, cw1
            for n in range(8):
                pc = ps.tile([64, 512], f32, tag="ps")
                nc.tensor.matmul(pc[:], cw1b[:], relTb[:, 512 * n:512 * (n + 1)],
                                 start=True, stop=True)
                nc.scalar.activation(h_relu[:, 512 * n:512 * (n + 1)], pc[:],
                                     Relu, bias=cb1[:])
            cw2m = singles.tile([64, 8], dt_mid, tag="cw2m")
            nc.vector.tensor_copy(cw2m[:], cw2[:])
            h3 = h_relu.rearrange("e (i j) -> e i j", j=64)
            b3 = bias_sb.rearrange("i (h j) -> i h j", j=64)
            for j in range(64):
                pb = ps.tile([64, 8], f32, tag="ps")
                nc.tensor.matmul(pb[:], h3[:, :, j], cw2m[:], start=True, stop=True)
                nc.vector.tensor_copy(b3[:, :, j], pb[:])

            # ---- phase 2: QKV mixing -> psum_q/k/v [s, (jm, h, d)] ----
            wq3 = wqrm.rearrange("c (o m) -> c o m", m=4)
            wi3 = wqim.rearrange("c (o m) -> c o m", m=4)
            win3 = wqin.rearrange("c (o m) -> c o m", m=4)
            pq = psl.tile([64, 512], f32, tag="psq")
            pk = psl.tile([64, 512], f32, tag="psk")
            pv = psl.tile([64, 512], f32, tag="psv")
            for m in range(4):
                lR = XR[:, 64 * m:64 * (m + 1)]
                lI = XI[:, 64 * m:64 * (m + 1)]
                for dst, o0 in ((pq, 0), (pk, 64), (pv, 128)):
                    wR = wq3[:, o0:o0 + 64, m]
                    wI = wi3[:, o0:o0 + 64, m]
                    wIn = win3[:, o0:o0 + 64, m]
                    blk = dst[:, 64 * (2 * m):64 * (2 * m) + 64]
                    nc.tensor.matmul(blk, lR, wR, start=True, stop=False)
                    nc.tensor.matmul(blk, lI, wIn, start=False, stop=True)
                    blk = dst[:, 64 * (2 * m + 1):64 * (2 * m + 1) + 64]
                    nc.tensor.matmul(blk, lR, wI, start=True, stop=False)
                    nc.tensor.matmul(blk, lI, wR, start=False, stop=True)
            nc.vector.tensor_copy(Q_sb[:], pq[:])
            nc.scalar.copy(K_sb[:], pk[:])
            nc.vector.tensor_copy(V_sb[:], pv[:])

            # ---- attention per head ----
            q4 = Q_sb.rearrange("s (j h d) -> s j h d", h=8, d=8)
            k4 = K_sb.rearrange("s (j h d) -> s j h d", h=8, d=8)
            v4 = V_sb.rearrange("s (j h d) -> s j h d", h=8, d=8)
            o4 = O_all.rearrange("s (j h d) -> s j h d", h=8, d=8)
            for h in range(8):
                qhs = sm_pool.tile([64, 64], dt_mid, tag="qhs")
                nc.vector.tensor_copy(qhs[:], q4[:, :, h, :])
                tid2 = identb if dt_mid != f32 else ident
                ptq = ps.tile([64, 64], dt_mid, tag="ps")
                nc.tensor.transpose(ptq[:], qhs[:], tid2[:64, :64])
                qh = sm_pool.tile([64, 64], dt_mid, tag="qh")
                nc.vector.tensor_scalar_mul(qh[:], ptq[:], gcol[:])
                khs = sm_pool.tile([64, 64], dt_mid, tag="khs")
                nc.scalar.copy(khs[:], k4[:, :, h, :])
                ptk = ps.tile([64, 64], dt_mid, tag="ps")
                nc.tensor.transpose(ptk[:], khs[:], tid2[:64, :64])
                kh = sm_pool.tile([64, 64], dt_mid, tag="kh")
                nc.scalar.copy(kh[:], ptk[:])
                pss = ps.tile([64, 64], f32, tag="ps")
                nc.tensor.matmul(pss[:], qh[:], kh[:], start=True, stop=True)
                ex = sm_pool.tile([64, 64], f32, tag="ex")
                sc = sm_pool.tile([64, 64], f32, tag="sc")
                nc.vector.tensor_add(sc[:], pss[:], bias_sb[:, 64 * h:64 * h + 64])
                nc.scalar.activation(ex[:], sc[:], Exp)
                se = sm_pool.tile([64, 1], f32, tag="se")
                nc.vector.reduce_sum(se[:], ex[:], axis=mybir.AxisListType.X)
                ri = sm_pool.tile([64, 1], f32, tag="ri")
                nc.vector.reciprocal(ri[:], se[:])
                an = sm_pool.tile([64, 64], dt_mid, tag="an")
                nc.vector.tensor_scalar_mul(an[:], ex[:], ri[:])
                pat = ps.tile([64, 64], dt_mid, tag="ps")
                nc.tensor.transpose(pat[:], an[:], tid2[:64, :64])
                at = sm_pool.tile([64, 64], dt_mid, tag="at")
                nc.scalar.copy(at[:], pat[:])
                vh = sm_pool.tile([64, 64], dt_mid, tag="vh")
                nc.vector.tensor_copy(vh[:], v4[:, :, h, :])
                po = ps.tile([64, 64], f32, tag="ps")
                nc.tensor.matmul(po[:], at[:], vh[:], start=True, stop=True)
                nc.vector.tensor_copy(o4[:, :, h, :], po[:])

            # ---- phase 6: T8 scale, transpose, w_out mixing ----
            O_scm = O_sc if dt_mid == f32 else singles.tile([64, 512], dt_mid, tag="Oscm")
            nc.vector.tensor_mul(O_scm[:], O_all[:], t8rep[:])
            xor3 = XOR.rearrange("c (m s) -> c m s", s=64)
            xoi3 = XOI.rearrange("c (m s) -> c m s", s=64)
            for jm in range(8):
                pt = ps.tile([64, 64], dt_mid, tag="ps")
                tid3 = identb if dt_mid != f32 else ident
                nc.tensor.transpose(pt[:], O_scm[:, 64 * jm:64 * (jm + 1)],
                                    tid3[:64, :64])
                dst3 = xor3 if jm % 2 == 0 else xoi3
                if jm % 2 == 0:
                    nc.vector.tensor_copy(dst3[:, jm // 2, :], pt[:])
                else:
                    nc.scalar.copy(dst3[:, jm // 2, :], pt[:])
            wo3 = worm.rearrange("c (o m) -> c o m", m=4)
            woi3_ = woim.rearrange("c (o m) -> c o m", m=4)
            woin3 = woin.rearrange("c (o m) -> c o m", m=4)
            pf = psl.tile([64, 512], f32, tag="psq")
            for m in range(4):
                rR = XOR[:, 64 * m:64 * (m + 1)]
                rI = XOI[:, 64 * m:64 * (m + 1)]
                wR = wo3[:, :, m]
                wI = woi3_[:, :, m]
                wIn = woin3[:, :, m]
                blk = pf[:, 64 * (2 * m):64 * (2 * m) + 64]
                nc.tensor.matmul(blk, wR, rR, start=True, stop=False)
                nc.tensor.matmul(blk, wIn, rI, start=False, stop=True)
                blk = pf[:, 64 * (2 * m + 1):64 * (2 * m + 1) + 64]
                nc.tensor.matmul(blk, wI, rR, start=True, stop=False)
                nc.tensor.matmul(blk, wR, rI, start=False, stop=True)
            nc.vector.tensor_copy(F_sb[:], pf[:])

            # ---- build U_fT [8, rows] ----
            f3 = F_sb.rearrange("c (j s) -> c j s", s=64)
            for s in range(64):
                pu = ps.tile([8, 64], f32, tag="ps")
                nc.tensor.transpose(pu[:], f3[:, :, s], ident[:64, :64])
                if s % 2 == 0:
                    nc.vector.tensor_copy(U_fT[:, 64 * s:64 * (s + 1)], pu[:])
                else:
                    nc.scalar.copy(U_fT[:, 64 * s:64 * (s + 1)], pu[:])

            # ---- stage 7: y = U_fT^T @ Bas, stream out ----
            for t in range(32):
                lh = U_fT[:, 128 * t:128 * (t + 1)]
                py1 = ps.tile([128, 512], f32, tag="ps")
                nc.tensor.matmul(py1[:], lh, basb[:, :512], start=True, stop=True)
                py2 = ps.tile([128, 512], f32, tag="ps")
                nc.tensor.matmul(py2[:], lh, basb[:, 512:], start=True, stop=True)
                y_sb = y_pool.tile([128, 1024], mybir.dt.float16, tag="y_sb")
                nc.vector.tensor_copy(y_sb[:, :512], py1[:])
                nc.scalar.copy(y_sb[:, 512:], py2[:])
                nc.sync.dma_start(out=y_out[128 * t:128 * (t + 1), :], in_=y_sb[:])
    nc.finalize()
    return nc


_NC_CACHE = {}


def kernel(**inputs) -> np.ndarray:
    from concourse.bass_utils import run_bass_kernel_spmd

    seq = np.asarray(inputs["seq"], dtype=np.float32)
    assert seq.shape == (B, S, C, H, W)
    e8c, bas, relT, gcol, t8pat = _constants()

    if DT_BIG not in _NC_CACHE:
        _NC_CACHE[DT_BIG] = _build(DT_BIG)
    nc = _NC_CACHE[DT_BIG]

    common = {
        "wqr": np.ascontiguousarray(np.asarray(inputs["w_qkv_r"], np.float32).reshape(64, 768)),
        "wqi": np.ascontiguousarray(np.asarray(inputs["w_qkv_i"], np.float32).reshape(64, 768)),
        "wor": np.ascontiguousarray(np.asarray(inputs["w_out_r"], np.float32).reshape(64, 256)),
        "woi": np.ascontiguousarray(np.asarray(inputs["w_out_i"], np.float32).reshape(64, 256)),
        "cw1": np.asarray(inputs["cpb_w1"], np.float32),
        "cb1": np.asarray(inputs["cpb_b1"], np.float32).reshape(64, 1),
        "cw2": np.asarray(inputs["cpb_w2"], np.float32),
        "e8c": e8c, "bas": bas, "relT": relT, "gcol": gcol, "t8pat": t8pat,
    }
    in_maps = []
    for b in range(NCORES):
        m = dict(common)
        m["x"] = np.ascontiguousarray(seq[b].reshape(4096, 1024))
        in_maps.append(m)

    res = run_bass_kernel_spmd(nc, in_maps, list(range(NCORES)))
    out = np.stack([res.results[b]["y"].astype(np.float32).reshape(S, C, H, W)
                    for b in range(NCORES)])
    return out



# revision 10
# speedup vs baseline: 1.3251x; 1.3251x over previous
"""Trainium2 Bass kernel for nn_FuncSelfAttention (spectral self-attention).

v2 design (cost-model driven):
  - Host casts seq to f16; x arrives in DRAM as [4096, 1024] f16 per core
    (data-parallel over batch, 1 batch element per core).
  - x^T loaded via 8 hardware DMA-transposes (xbar, 14ns/tile) straight into
    SBUF as [128 hw, 4096 (s,c)] chunks -- no PE transposes, no PSUM copies.
  - Projection to 2x2 Fourier modes: 512 tiny matmuls (out [64c, 8jm] per s,
    accumulated over the 8 hw-chunks) into ONE psum bank -> Xall [c,(s,jm)].
  - QKV complex mixing, per-head attention with transposed-attention trick
    (scores->exp->normalize->transpose once), phase-6 output assembled
    directly in [c=(h,d), (m,s)] layout by 64 tiny matmuls.
  - w_out complex mixing -> F [c_out,(jm,s)]; U_fT [8,4096] via 64 cheap f16
    PE transposes; stage 7 streams y = U_fT^T @ Bas in 32 chunks of
    [128,1024]: PE matmul (f32 psum) -> one DVE/Act/Pool cast-copy -> DMA out
    (f16), round-robined across queues/engines.
"""
import numpy as np

B, S, C, H, W = 8, 64, 64, 32, 32
NH, HD = 8, 8
HW = H * W
NCORES = 8
MODES4 = [(0, 0), (0, 1), (1, 0), (1, 1)]


def _constants():
    hh, ww = np.meshgrid(np.arange(H), np.arange(W), indexing="ij")
    phi, psi = 2 * np.pi / H, 2 * np.pi / W
    E8 = np.zeros((HW, 8))
    Bas = np.zeros((8, HW))
    for mi, (kx, ky) in enumerate(MODES4):
        th = phi * kx * hh + psi * ky * ww
        E8[:, 2 * mi] = np.cos(th).ravel()
        E8[:, 2 * mi + 1] = -np.sin(th).ravel()
        mult = 1.0 if ky == 0 else 2.0
        Bas[2 * mi] = mult / HW * np.cos(th).ravel()
        Bas[2 * mi + 1] = -mult / HW * np.sin(th).ravel()
    g = (Bas @ Bas.T).diagonal().copy()      # attention Gram diag (per jm)
    t8d = (Bas @ E8).diagonal().copy()       # coeff->mode map (diagonal)

    e8c = np.zeros((128, 64), np.float32)    # hw-chunk k of E8 at cols [8k,8k+8)
    for k in range(8):
        e8c[:, 8 * k:8 * k + 8] = E8[128 * k:128 * (k + 1)]

    gx, gy = np.meshgrid(np.arange(8), np.arange(8), indexing="ij")
    coords = np.stack([gx.ravel(), gy.ravel()], -1).astype(np.float32)
    rel = coords[:, None, :] - coords[None, :, :]
    rel = np.sign(rel) * np.log2(1.0 + np.abs(rel))          # [64, 64, 2]
    relT = np.ascontiguousarray(rel.reshape(4096, 2).T).astype(np.float32)

    scale = np.float32(1.0 / HW) / np.float32(np.sqrt(HD))
    gcol = np.zeros((64, 1), np.float32)     # rows (jm, d): p = jm*8+d
    for p in range(64):
        gcol[p, 0] = g[p // 8] * scale
    t8pat = np.zeros((1, 512), np.float32)   # over (h, jm, d): col = h*64+jm*8+d
    for col in range(512):
        t8pat[0, col] = t8d[(col // 8) % 8]
    return e8c, Bas.astype(np.float32), relT, gcol, t8pat


def _build():
    import concourse.bass as bass
    import concourse.mybir as mybir
    import concourse.tile as tile
    from concourse import bacc

    f32 = mybir.dt.float32
    f16 = mybir.dt.float16
    Exp = mybir.ActivationFunctionType.Exp
    Relu = mybir.ActivationFunctionType.Relu

    nc = bacc.Bacc("TRN2", target_bir_lowering=False, debug=False,
                   dynamic_dma_scratch_size=49152)
    x_in = nc.dram_tensor("x", [4096, 1024], f16, kind="ExternalInput")
    wqr_in = nc.dram_tensor("wqr", [64, 768], f16, kind="ExternalInput")
    wqi_in = nc.dram_tensor("wqi", [64, 768], f16, kind="ExternalInput")
    wqin_in = nc.dram_tensor("wqin", [64, 768], f16, kind="ExternalInput")
    wor_in = nc.dram_tensor("wor", [64, 256], f16, kind="ExternalInput")
    woi_in = nc.dram_tensor("woi", [64, 256], f16, kind="ExternalInput")
    woin_in = nc.dram_tensor("woin", [64, 256], f16, kind="ExternalInput")
    cw1_in = nc.dram_tensor("cw1", [2, 64], f16, kind="ExternalInput")
    cb1_in = nc.dram_tensor("cb1", [64, 1], f32, kind="ExternalInput")
    cw2_in = nc.dram_tensor("cw2", [64, 8], f16, kind="ExternalInput")
    e8_in = nc.dram_tensor("e8c", [128, 64], f16, kind="ExternalInput")
    bas_in = nc.dram_tensor("bas", [8, 1024], f16, kind="ExternalInput")
    rel_in = nc.dram_tensor("relT", [2, 4096], f16, kind="ExternalInput")
    gcol_in = nc.dram_tensor("gcol", [64, 1], f32, kind="ExternalInput")
    t8_in = nc.dram_tensor("t8pat", [1, 512], f32, kind="ExternalInput")
    y_out = nc.dram_tensor("y", [4096, 1024], f16, kind="ExternalOutput")

    with tile.TileContext(nc) as tc:
        import contextlib
        ctx = contextlib.ExitStack()
        with ctx:
            singles = ctx.enter_context(tc.tile_pool(name="singles", bufs=1))
            psm = ctx.enter_context(tc.tile_pool(name="psm", bufs=3, space="PSUM"))
            ps7 = ctx.enter_context(tc.tile_pool(name="ps7", bufs=5, space="PSUM"))
            y_pool = ctx.enter_context(tc.tile_pool(name="yp", bufs=6))

            # ---- constants / weights into SBUF (tiny DMAs first) ----
            def load1(name, dram, shape, dt):
                t = singles.tile(shape, dt, tag=name)
                nc.scalar.dma_start(out=t[:], in_=dram[:])
                return t

            e8 = load1("e8", e8_in, [128, 64], f16)
            bas = load1("bas", bas_in, [8, 1024], f16)
            relT = load1("relT", rel_in, [2, 4096], f16)
            gcol = load1("gcol", gcol_in, [64, 1], f32)
            wqr = load1("wqr", wqr_in, [64, 768], f16)
            wqi = load1("wqi", wqi_in, [64, 768], f16)
            wqin = load1("wqin", wqin_in, [64, 768], f16)
            wor = load1("wor", wor_in, [64, 256], f16)
            woi = load1("woi", woi_in, [64, 256], f16)
            woin = load1("woin", woin_in, [64, 256], f16)
            cw1 = load1("cw1", cw1_in, [2, 64], f16)
            cb1 = load1("cb1", cb1_in, [64, 1], f32)
            cw2 = load1("cw2", cw2_in, [64, 8], f16)
            t8rep = singles.tile([64, 512], f32, tag="t8rep")
            nc.scalar.dma_start(out=t8rep[:], in_=t8_in[:].to_broadcast([64, 512]))

            # ---- x^T via hardware DMA transpose: 8 chunks [128, 4096] ----
            xT = []
            for k in range(8):
                t = singles.tile([128, 4096], f16, tag=f"xT{k}")
                eng = (nc.sync, nc.scalar)[k % 2]
                eng.dma_start_transpose(t[:], x_in[:, 128 * k:128 * (k + 1)])
                xT.append(t)

            # ---- CPB bias (independent of x; overlaps the xT loads) ----
            # h_relu [e=64, (i,j)=4096] = relu(cw1^T @ relT + b1)
            h_relu = singles.tile([64, 4096], f16, tag="hrelu")
            for n in range(8):
                pc = psm.tile([64, 512], f32, tag="m")
                nc.tensor.matmul(pc[:], cw1[:], relT[:, 512 * n:512 * (n + 1)],
                                 start=True, stop=True)
                nc.scalar.activation(h_relu[:, 512 * n:512 * (n + 1)], pc[:],
                                     Relu, bias=cb1[:])
            # bias [i, (j, h)]: 64 matmuls into one psum bank, one copy out
            h3 = h_relu.rearrange("e (i j) -> e i j", j=64)
            pb = psm.tile([64, 512], f32, tag="m")
            for j in range(64):
                nc.tensor.matmul(pb[:, 8 * j:8 * j + 8], h3[:, :, j], cw2[:],
                                 start=True, stop=True)
            bias_jh = singles.tile([64, 512], f32, tag="bias_jh")  # [i, (j, h)]
            nc.vector.tensor_copy(bias_jh[:], pb[:])

            # ---- projection: Xall [c, (s, jm)] f16 ----
            # per s: psum [64, 8] at col 8s accumulates over 8 hw-chunks
            pX = psm.tile([64, 512], f32, tag="m")
            for s in range(64):
                for k in range(8):
                    nc.tensor.matmul(pX[:, 8 * s:8 * s + 8],
                                     xT[k][:, 64 * s:64 * (s + 1)],
                                     e8[:, 8 * k:8 * k + 8],
                                     start=(k == 0), stop=(k == 7))
            Xsj = singles.tile([64, 512], f16, tag="Xsj")   # [c, (s, jm)]
            nc.vector.tensor_copy(Xsj[:], pX[:])
            # re-layout to [c, (jm, s)] for contiguous lhsT slices
            Xjs = singles.tile([64, 512], f16, tag="Xjs")   # [c, (jm, s)]
            src = Xsj.rearrange("c (s j) -> c j s", j=8)
            dst = Xjs.rearrange("c (j s) -> c j s", s=64)
            nc.vector.tensor_copy(dst[:], src[:])

            # ---- QKV complex mixing -> Q/K/V [s, (jm, h, d)] ----
            wq3 = wqr.rearrange("c (o m) -> c o m", m=4)
            wi3 = wqi.rearrange("c (o m) -> c o m", m=4)
            win3 = wqin.rearrange("c (o m) -> c o m", m=4)
            xj3 = Xjs.rearrange("c (j s) -> c j s", s=64)
            Q_sb = singles.tile([64, 512], f16, tag="Qsb")
            K_sb = singles.tile([64, 512], f16, tag="Ksb")
            V_sb = singles.tile([64, 512], f16, tag="Vsb")
            for qi, (sb, o0) in enumerate(((Q_sb, 0), (K_sb, 64), (V_sb, 128))):
                pd = psm.tile([64, 512], f32, tag="m")
                for m in range(4):
                    lR = xj3[:, 2 * m, :]
                    lI = xj3[:, 2 * m + 1, :]
                    wR = wq3[:, o0:o0 + 64, m]
                    wI = wi3[:, o0:o0 + 64, m]
                    wIn = win3[:, o0:o0 + 64, m]
                    blk = pd[:, 64 * (2 * m):64 * (2 * m) + 64]
                    nc.tensor.matmul(blk, lR, wR, start=True, stop=False)
                    nc.tensor.matmul(blk, lI, wIn, start=False, stop=True)
                    blk = pd[:, 64 * (2 * m + 1):64 * (2 * m + 1) + 64]
                    nc.tensor.matmul(blk, lR, wI, start=True, stop=False)
                    nc.tensor.matmul(blk, lI, wR, start=False, stop=True)
                pdv = pd.rearrange("s (j h d) -> s j h d", h=8, d=8)
                sbv = sb.rearrange("s (h j d) -> s j h d", j=8, d=8)
                if qi == 0:
                    nc.vector.tensor_copy(sbv[:], pdv[:])
                elif qi == 1:
                    nc.scalar.copy(sbv[:], pdv[:])
                else:
                    # V scaled by t8 diag (fold of the coeff->mode map);
                    # t8rep pattern is laid out for the (h, jm, d) output order
                    nc.vector.tensor_mul(sbv[:], pdv[:],
                                         t8rep.rearrange("s (h j d) -> s j h d",
                                                         j=8, d=8)[:])

            # ---- attention ----
            ident = singles.tile([64, 64], f16, tag="ident")
            from concourse.masks import make_identity
            make_identity(nc, ident[:])

            # per-head transposes of Q and K into batched psum, then one
            # scaled copy each: QT/KT [ (jm,d), (h, s) ]
            QT = singles.tile([64, 512], f16, tag="QT")
            KT = singles.tile([64, 512], f16, tag="KT")
            pqt = psm.tile([64, 512], f16, tag="m")
            for h in range(8):
                nc.tensor.transpose(pqt[:, 64 * h:64 * (h + 1)],
                                    Q_sb[:, 64 * h:64 * (h + 1)], ident[:])
            nc.vector.tensor_scalar_mul(QT[:], pqt[:], gcol[:])
            pkt = psm.tile([64, 512], f16, tag="m")
            for h in range(8):
                nc.tensor.transpose(pkt[:, 64 * h:64 * (h + 1)],
                                    K_sb[:, 64 * h:64 * (h + 1)], ident[:])
            nc.scalar.copy(KT[:], pkt[:])

            # scores per head into one psum bank [i, (h, j)]
            pS = psm.tile([64, 512], f32, tag="m")
            for h in range(8):
                nc.tensor.matmul(pS[:, 64 * h:64 * (h + 1)],
                                 QT[:, 64 * h:64 * (h + 1)],
                                 KT[:, 64 * h:64 * (h + 1)],
                                 start=True, stop=True)
            # add bias (strided view of [i,(j,h)] -> [i,(h,j)]) and exp
            sc_sb = singles.tile([64, 512], f32, tag="sc")
            bview = bias_jh.rearrange("i (j h) -> i h j", h=8)
            sview = sc_sb.rearrange("i (h j) -> i h j", j=64)
            nc.vector.tensor_add(sview[:], pS.rearrange("i (h j) -> i h j", j=64)[:],
                                 bview[:])
            ex = singles.tile([64, 512], f32, tag="ex")
            nc.scalar.activation(ex[:], sc_sb[:], Exp)
            se = singles.tile([64, 8], f32, tag="se")
            nc.vector.reduce_sum(se[:], ex.rearrange("i (h j) -> i h j", j=64)[:],
                                 axis=mybir.AxisListType.X)
            ri = singles.tile([64, 8], f32, tag="ri")
            nc.vector.reciprocal(ri[:], se[:])
            # normalized attention (f16) [i, (h, j)]
            an = singles.tile([64, 512], f16, tag="an")
            for h in range(8):
                nc.vector.tensor_scalar_mul(an[:, 64 * h:64 * (h + 1)],
                                            ex[:, 64 * h:64 * (h + 1)],
                                            ri[:, h:h + 1])
            # transpose attention per head -> amT [j, (h, i)]
            pat = psm.tile([64, 512], f16, tag="m")
            for h in range(8):
                nc.tensor.transpose(pat[:, 64 * h:64 * (h + 1)],
                                    an[:, 64 * h:64 * (h + 1)], ident[:])
            amT = singles.tile([64, 512], f16, tag="amT")
            nc.vector.tensor_copy(amT[:], pat[:])

            # ---- attn @ V -> O [i, (h, jm, d)], then per-jm transposes ----
            pO = psm.tile([64, 512], f32, tag="m")
            for h in range(8):
                nc.tensor.matmul(pO[:, 64 * h:64 * (h + 1)],
                                 amT[:, 64 * h:64 * (h + 1)],
                                 V_sb[:, 64 * h:64 * (h + 1)],
                                 start=True, stop=True)
            O_sb = singles.tile([64, 512], f16, tag="Osb")  # [i, (jm, h, d)]
            nc.vector.tensor_copy(
                O_sb.rearrange("i (j h d) -> i j h d", h=8, d=8)[:],
                pO.rearrange("i (h j d) -> i j h d", j=8, d=8)[:])
            # transpose [s, (h,d)] slice per jm -> [ (h,d), s ]; XOR | XOI
            pXO = psm.tile([64, 512], f16, tag="m")
            for m in range(4):
                nc.tensor.transpose(pXO[:, 64 * m:64 * (m + 1)],
                                    O_sb[:, 128 * m:128 * m + 64], ident[:])
                nc.tensor.transpose(pXO[:, 256 + 64 * m:256 + 64 * (m + 1)],
                                    O_sb[:, 128 * m + 64:128 * (m + 1)], ident[:])
            XO = singles.tile([64, 512], f16, tag="XO")
            nc.vector.tensor_copy(XO[:], pXO[:])
            XOR = XO[:, 0:256].rearrange("c (m s) -> c m s", s=64)
            XOI = XO[:, 256:512].rearrange("c (m s) -> c m s", s=64)

            # ---- w_out complex mixing -> F [c_out, (jm, s)] ----
            wo3 = wor.rearrange("c (o m) -> c o m", m=4)
            woi3 = woi.rearrange("c (o m) -> c o m", m=4)
            woin3 = woin.rearrange("c (o m) -> c o m", m=4)
            pf = psm.tile([64, 512], f32, tag="m")
            for m in range(4):
                rR = XOR[:, m, :]
                rI = XOI[:, m, :]
                blk = pf[:, 64 * (2 * m):64 * (2 * m) + 64]
                nc.tensor.matmul(blk, wo3[:, :, m], rR, start=True, stop=False)
                nc.tensor.matmul(blk, woin3[:, :, m], rI, start=False, stop=True)
                blk = pf[:, 64 * (2 * m + 1):64 * (2 * m + 1) + 64]
                nc.tensor.matmul(blk, woi3[:, :, m], rR, start=True, stop=False)
                nc.tensor.matmul(blk, wo3[:, :, m], rI, start=False, stop=True)
            F_sb = singles.tile([64, 512], f16, tag="Fsb")
            nc.vector.tensor_copy(F_sb[:], pf[:])

            # ---- U_fT [8, (s, c)] via 64 cheap f16 transposes ----
            f3 = F_sb.rearrange("c (j s) -> c j s", s=64)
            U_fT = singles.tile([8, 4096], f16, tag="UfT")
            for g8 in range(8):
                pu = psm.tile([8, 512], f16, tag="m")
                for u in range(8):
                    s = 8 * g8 + u
                    nc.tensor.transpose(pu[:, 64 * u:64 * (u + 1)],
                                        f3[:, :, s], ident[:])
                if g8 % 2 == 0:
                    nc.vector.tensor_copy(U_fT[:, 512 * g8:512 * (g8 + 1)], pu[:])
                else:
                    nc.scalar.copy(U_fT[:, 512 * g8:512 * (g8 + 1)], pu[:])

            # ---- stage 7: y = U_fT^T @ Bas, streamed in 32 chunks ----
            cp_engines = [nc.vector, nc.scalar]
            dma_engines = [nc.sync, nc.scalar]
            for t in range(32):
                lh = U_fT[:, 128 * t:128 * (t + 1)]
                py0 = ps7.tile([128, 512], f32, tag="y")
                py1 = ps7.tile([128, 512], f32, tag="y")
                nc.tensor.matmul(py0[:], lh, bas[:, :512], start=True, stop=True)
                nc.tensor.matmul(py1[:], lh, bas[:, 512:], start=True, stop=True)
                y_sb = y_pool.tile([128, 1024], f16, tag="y_sb")
                for half, py in ((0, py0), (1, py1)):
                    ce = cp_engines[(2 * t + half) % 2]
                    dstv = y_sb[:, 512 * half:512 * (half + 1)]
                    if ce is nc.scalar:
                        ce.copy(dstv, py[:])
                    else:
                        ce.tensor_copy(dstv, py[:])
                dma_engines[t % 2].dma_start(
                    out=y_out[128 * t:128 * (t + 1), :], in_=y_sb[:])
    nc.finalize()
    return nc


_NC_CACHE = {}


def kernel(**inputs) -> np.ndarray:
    from concourse.bass_utils import run_bass_kernel_spmd

    seq = np.asarray(inputs["seq"], dtype=np.float32)
    assert seq.shape == (B, S, C, H, W)
    e8c, bas, relT, gcol, t8pat = _constants()

    if "nc" not in _NC_CACHE:
        _NC_CACHE["nc"] = _build()
    nc = _NC_CACHE["nc"]

    wqr = np.asarray(inputs["w_qkv_r"], np.float32).reshape(64, 768)
    wqi = np.asarray(inputs["w_qkv_i"], np.float32).reshape(64, 768)
    wor = np.asarray(inputs["w_out_r"], np.float32).reshape(64, 256)
    woi = np.asarray(inputs["w_out_i"], np.float32).reshape(64, 256)
    common = {
        "wqr": np.ascontiguousarray(wqr).astype(np.float16),
        "wqi": np.ascontiguousarray(wqi).astype(np.float16),
        "wqin": np.ascontiguousarray(-wqi).astype(np.float16),
        "wor": np.ascontiguousarray(wor).astype(np.float16),
        "woi": np.ascontiguousarray(woi).astype(np.float16),
        "woin": np.ascontiguousarray(-woi).astype(np.float16),
        "cw1": np.asarray(inputs["cpb_w1"], np.float32).astype(np.float16),
        "cb1": np.asarray(inputs["cpb_b1"], np.float32).reshape(64, 1),
        "cw2": np.asarray(inputs["cpb_w2"], np.float32).astype(np.float16),
        "e8c": e8c.astype(np.float16),
        "bas": bas.astype(np.float16),
        "relT": relT.astype(np.float16),
        "gcol": gcol, "t8pat": t8pat,
    }
    seq16 = seq.reshape(B, 4096, 1024).astype(np.float16)
    in_maps = []
    for b in range(NCORES):
        m = dict(common)
        m["x"] = seq16[b]
        in_maps.append(m)

    res = run_bass_kernel_spmd(nc, in_maps, list(range(NCORES)))
    out = np.stack([res.results[b]["y"].astype(np.float32).reshape(S, C, H, W)
                    for b in range(NCORES)])
    return out


# revision 20
# speedup vs baseline: 1.7385x; 1.3120x over previous
"""Trainium2 Bass kernel for nn_FuncSelfAttention (spectral self-attention).

v3 design (cost-model driven):
  - Host casts seq to f16 AND pre-transposes it: x^T [1024 hw, 4096 (s,c)]
    per core (data-parallel over batch, 1 batch element per core). Device
    does 8 plain 1MB DMA loads (no transpose premium, no queue clog).
  - All weights/constants packed host-side into 3 DMAs (ring depth per
    HWDGE queue is 2, so many small loads would serialize at ~1 per big
    transfer otherwise).
  - Projection to 2x2 Fourier modes: 512 tiny matmuls (out [64c, 8jm] per s,
    accumulated over the 8 hw-chunks) into ONE psum bank -> Xall [c,(s,jm)].
  - QKV complex mixing, batched-head attention (one exp/reduce/recip pass),
    attn@V per head, per-jm transposes, w_out mixing, U_fT via 64 f16
    transposes; stage 7 streams y = U_fT^T @ Bas in 32 chunks of [128,1024]:
    PE matmul (f32 psum) -> DVE/Act half-copies (cast f16) -> DMA out on SP.
"""
import numpy as np

B, S, C, H, W = 8, 64, 64, 32, 32
NH, HD = 8, 8
HW = H * W
NCORES = 8
MODES4 = [(0, 0), (0, 1), (1, 0), (1, 1)]

MAIN_COLS = 4106


def _constants():
    hh, ww = np.meshgrid(np.arange(H), np.arange(W), indexing="ij")
    phi, psi = 2 * np.pi / H, 2 * np.pi / W
    E8 = np.zeros((HW, 8))
    Bas = np.zeros((8, HW))
    for mi, (kx, ky) in enumerate(MODES4):
        th = phi * kx * hh + psi * ky * ww
        E8[:, 2 * mi] = np.cos(th).ravel()
        E8[:, 2 * mi + 1] = -np.sin(th).ravel()
        mult = 1.0 if ky == 0 else 2.0
        Bas[2 * mi] = mult / HW * np.cos(th).ravel()
        Bas[2 * mi + 1] = -mult / HW * np.sin(th).ravel()
    g = (Bas @ Bas.T).diagonal().copy()      # attention Gram diag (per jm)
    t8d = (Bas @ E8).diagonal().copy()       # coeff->mode map (diagonal)

    e8c = np.zeros((128, 64), np.float32)    # hw-chunk k of E8 at cols [8k,8k+8)
    for k in range(8):
        e8c[:, 8 * k:8 * k + 8] = E8[128 * k:128 * (k + 1)]

    gx, gy = np.meshgrid(np.arange(8), np.arange(8), indexing="ij")
    coords = np.stack([gx.ravel(), gy.ravel()], -1).astype(np.float32)
    rel = coords[:, None, :] - coords[None, :, :]
    rel = np.sign(rel) * np.log2(1.0 + np.abs(rel))          # [64, 64, 2]
    relT = np.ascontiguousarray(rel.reshape(4096, 2).T).astype(np.float32)

    scale = np.float32(1.0 / HW) / np.float32(np.sqrt(HD))
    gcol = np.zeros((64, 1), np.float32)     # rows (jm, d): p = jm*8+d
    for p in range(64):
        gcol[p, 0] = g[p // 8] * scale
    t8pat = np.zeros((64, 512), np.float32)  # over (h, jm, d): col = h*64+jm*8+d
    for col in range(512):
        t8pat[:, col] = t8d[(col // 8) % 8]
    return e8c, Bas.astype(np.float32), relT, gcol, t8pat


def _pack_weights(inputs):
    """mainpack [128, MAIN_COLS] f16, relTpack [2, 4160] f16, bas [8,1024] f16."""
    e8c, bas, relT, gcol, t8pat = _constants()
    wqr = np.asarray(inputs["w_qkv_r"], np.float32).reshape(64, 768)
    wqi = np.asarray(inputs["w_qkv_i"], np.float32).reshape(64, 768)
    wor = np.asarray(inputs["w_out_r"], np.float32).reshape(64, 256)
    woi = np.asarray(inputs["w_out_i"], np.float32).reshape(64, 256)
    cw1 = np.asarray(inputs["cpb_w1"], np.float32)
    cb1 = np.asarray(inputs["cpb_b1"], np.float32).reshape(64, 1)
    cw2 = np.asarray(inputs["cpb_w2"], np.float32)

    main = np.zeros((64, MAIN_COLS), np.float16)
    main[:, 0:768] = wqr
    main[:, 768:1536] = wqi
    main[:, 1536:2304] = -wqi
    main[:, 2304:2560] = wor
    main[:, 2560:2816] = woi
    main[:, 2816:3072] = -woi
    main[:, 3072:3080] = cw2
    main[:, 3080:3081] = gcol
    main[:, 3081:3082] = cb1
    main[:, 3082:3594] = t8pat[:, :512]
    gpat = np.zeros((64, 512), np.float32)
    for p in range(64):
        gpat[p, :] = gcol[p, 0]
    main[:, 3594:4106] = gpat

    relTpack = np.zeros((2, 4160), np.float16)
    relTpack[:, 0:4096] = relT
    relTpack[:, 4096:4160] = cw1
    return e8c.astype(np.float16), main, relTpack, bas.astype(np.float16)


def _build(debug=False):
    import concourse.bass as bass
    import concourse.mybir as mybir
    import concourse.tile as tile
    from concourse import bacc
    from concourse.masks import make_identity

    f32 = mybir.dt.float32
    f16 = mybir.dt.float16
    Exp = mybir.ActivationFunctionType.Exp
    Relu = mybir.ActivationFunctionType.Relu

    nc = bacc.Bacc("TRN2", target_bir_lowering=False, debug=False,
                   dynamic_dma_scratch_size=49152)
    x_in = nc.dram_tensor("xt", [1024, 4096], f16, kind="ExternalInput")
    e8_in = nc.dram_tensor("e8pack", [128, 64], f16, kind="ExternalInput")
    main_in = nc.dram_tensor("mainpack", [64, MAIN_COLS], f16,
                             kind="ExternalInput")
    rel_in = nc.dram_tensor("relTpack", [2, 4160], f16, kind="ExternalInput")
    bas_in = nc.dram_tensor("bas", [8, 1024], f16, kind="ExternalInput")
    y_out = nc.dram_tensor("y", [4096, 1024], f16, kind="ExternalOutput")
    dbg_out = {}
    if debug:
        for nm in ("dXsj", "dQ", "dK", "dV", "dbias", "dan", "dU"):
            shp = [8, 4096] if nm == "dU" else [64, 512]
            dbg_out[nm] = nc.dram_tensor(nm, shp, f16, kind="ExternalOutput")

    with tile.TileContext(nc) as tc:
        import contextlib
        ctx = contextlib.ExitStack()
        with ctx:
            singles = ctx.enter_context(tc.tile_pool(name="singles", bufs=1))
            psm = ctx.enter_context(tc.tile_pool(name="psm", bufs=3, space="PSUM"))
            psX = ctx.enter_context(tc.tile_pool(name="psX", bufs=2, space="PSUM"))
            ps7 = ctx.enter_context(tc.tile_pool(name="ps7", bufs=3, space="PSUM"))
            y_pool = ctx.enter_context(tc.tile_pool(name="yp", bufs=6))

            # ---- packed constants: 4 DMAs on scalar; x chunks on sync ----
            relTp = singles.tile([2, 4160], f16, tag="relTp")
            nc.scalar.dma_start(out=relTp[:], in_=rel_in[:])
            e8t = singles.tile([128, 64], f16, tag="e8t")
            nc.scalar.dma_start(out=e8t[:], in_=e8_in[:])
            mainp = singles.tile([64, MAIN_COLS], f16, tag="mainp")
            nc.scalar.dma_start(out=mainp[:], in_=main_in[:])
            bas = singles.tile([8, 1024], f16, tag="bas")
            nc.scalar.dma_start(out=bas[:], in_=bas_in[:])

            xT = []
            for k in range(8):
                t = singles.tile([128, 4096], f16, tag=f"xT{k}")
                nc.sync.dma_start(out=t[:], in_=x_in[128 * k:128 * (k + 1), :])
                xT.append(t)

            wqr = mainp[:, 0:768]
            wqi = mainp[:, 768:1536]
            wqin = mainp[:, 1536:2304]
            wor = mainp[:, 2304:2560]
            woi = mainp[:, 2560:2816]
            woin = mainp[:, 2816:3072]
            cw2 = mainp[:, 3072:3080]
            gpat = mainp[:, 3594:4106]
            cb1 = mainp[:, 3081:3082]
            t8rep = mainp[:, 3082:3594]
            relT = relTp[:, 0:4096]
            cw1 = relTp[:, 4096:4160]

            ident = singles.tile([64, 64], f16, tag="ident")
            make_identity(nc, ident[:])

            # ---- projection (interleaved with CPB, which needs no x) ----
            # PSUM accumulation groups must be contiguous, so each chunk is a
            # single-shot matmul set into a rotating psum tile, accumulated
            # into SBUF f32 (last chunk writes the f16 result directly).
            Xacc = singles.tile([64, 512], f32, tag="Xacc")
            Xsj = singles.tile([64, 512], f16, tag="Xsj")   # [c, (s, jm)]

            def proj_chunk(k):
                pXk = psX.tile([64, 512], f32, tag="X")
                for s in range(64):
                    nc.tensor.matmul(pXk[:, 8 * s:8 * s + 8],
                                     xT[k][:, 64 * s:64 * (s + 1)],
                                     e8t[:, 8 * k:8 * k + 8],
                                     start=True, stop=True)
                if k == 0:
                    nc.vector.tensor_copy(Xacc[:], pXk[:])
                elif k < 7:
                    nc.vector.tensor_add(Xacc[:], pXk[:], Xacc[:])
                else:
                    nc.vector.tensor_add(Xsj[:], pXk[:], Xacc[:])

            proj_chunk(0)
            proj_chunk(1)
            # CPB layer 1: h_relu [e=64, (i,j)=4096] = relu(cw1^T @ relT + b1)
            h_relu = singles.tile([64, 4096], f16, tag="hrelu")
            for n in range(8):
                pc = psm.tile([64, 512], f32, tag="m")
                nc.tensor.matmul(pc[:], cw1[:], relT[:, 512 * n:512 * (n + 1)],
                                 start=True, stop=True)
                nc.scalar.activation(h_relu[:, 512 * n:512 * (n + 1)], pc[:],
                                     Relu, bias=cb1[:])
            proj_chunk(2)
            proj_chunk(3)
            # CPB layer 2: bias [i, (j, h)]: 64 matmuls, one copy out
            h3 = h_relu.rearrange("e (i j) -> e i j", j=64)
            pb = psm.tile([64, 512], f32, tag="m")
            for j in range(64):
                nc.tensor.matmul(pb[:, 8 * j:8 * j + 8], h3[:, :, j], cw2[:],
                                 start=True, stop=True)
            bias_jh = singles.tile([64, 512], f32, tag="bias_jh")  # [i, (j, h)]
            nc.vector.tensor_copy(bias_jh[:], pb[:])
            for k in range(4, 8):
                proj_chunk(k)

            Xjs = singles.tile([64, 512], f16, tag="Xjs")   # [c, (jm, s)]
            nc.vector.tensor_copy(
                Xjs.rearrange("c (j s) -> c j s", s=64)[:],
                Xsj.rearrange("c (s j) -> c j s", j=8)[:])
            xv = Xjs.rearrange("c (j s) -> c j s", s=64)

            # ---- QKV complex mixing -> Q/K/V [s, (h, jm, d)] ----
            wq3 = wqr.rearrange("c (o m) -> c o m", m=4)
            wi3 = wqi.rearrange("c (o m) -> c o m", m=4)
            win3 = wqin.rearrange("c (o m) -> c o m", m=4)
            Q_sb = singles.tile([64, 512], f16, tag="Qsb")
            K_sb = singles.tile([64, 512], f16, tag="Ksb")
            V_sb = singles.tile([64, 512], f16, tag="Vsb")
            for qi, (sb, o0) in enumerate(((Q_sb, 0), (K_sb, 64), (V_sb, 128))):
                pd = psm.tile([64, 512], f32, tag="m")
                for m in range(4):
                    lR = xv[:, 2 * m, :]
                    lI = xv[:, 2 * m + 1, :]
                    wR = wq3[:, o0:o0 + 64, m]
                    wI = wi3[:, o0:o0 + 64, m]
                    wIn = win3[:, o0:o0 + 64, m]
                    blk = pd[:, 64 * (2 * m):64 * (2 * m) + 64]
                    nc.tensor.matmul(blk, lR, wR, start=True, stop=False)
                    nc.tensor.matmul(blk, lI, wIn, start=False, stop=True)
                    blk = pd[:, 64 * (2 * m + 1):64 * (2 * m + 1) + 64]
                    nc.tensor.matmul(blk, lR, wI, start=True, stop=False)
                    nc.tensor.matmul(blk, lI, wR, start=False, stop=True)
                pdv = pd.rearrange("s (j h d) -> s j h d", h=8, d=8)
                sbv = sb.rearrange("s (h j d) -> s j h d", j=8, d=8)
                if qi == 0:
                    nc.vector.tensor_copy(sbv[:], pdv[:])
                elif qi == 1:
                    nc.scalar.copy(sbv[:], pdv[:])
                else:
                    # V scaled by t8 diag (fold of the coeff->mode map);
                    # t8rep pattern is laid out for the (h, jm, d) output order
                    nc.vector.tensor_mul(
                        sbv[:], pdv[:],
                        t8rep.rearrange("s (h j d) -> s j h d", j=8, d=8)[:])

            # ---- attention ----
            QT = singles.tile([64, 512], f16, tag="QT")
            KT = singles.tile([64, 512], f16, tag="KT")
            pqt = psm.tile([64, 512], f16, tag="m")
            for h in range(8):
                nc.tensor.transpose(pqt[:, 64 * h:64 * (h + 1)],
                                    Q_sb[:, 64 * h:64 * (h + 1)], ident[:])
            nc.vector.tensor_mul(QT[:], pqt[:], gpat[:])
            pkt = psm.tile([64, 512], f16, tag="m")
            for h in range(8):
                nc.tensor.transpose(pkt[:, 64 * h:64 * (h + 1)],
                                    K_sb[:, 64 * h:64 * (h + 1)], ident[:])
            nc.scalar.copy(KT[:], pkt[:])

            # scores per head into one psum bank [i, (h, j)]
            pS = psm.tile([64, 512], f32, tag="m")
            for h in range(8):
                nc.tensor.matmul(pS[:, 64 * h:64 * (h + 1)],
                                 QT[:, 64 * h:64 * (h + 1)],
                                 KT[:, 64 * h:64 * (h + 1)],
                                 start=True, stop=True)
            # add bias (strided view of [i,(j,h)] -> [i,(h,j)]) and exp
            sc_sb = singles.tile([64, 512], f32, tag="sc")
            bview = bias_jh.rearrange("i (j h) -> i h j", h=8)
            sview = sc_sb.rearrange("i (h j) -> i h j", j=64)
            nc.vector.tensor_add(sview[:],
                                 pS.rearrange("i (h j) -> i h j", j=64)[:],
                                 bview[:])
            ex = singles.tile([64, 512], f32, tag="ex")
            nc.scalar.activation(ex[:], sc_sb[:], Exp)
            se = singles.tile([64, 8], f32, tag="se")
            nc.vector.reduce_sum(se[:], ex.rearrange("i (h j) -> i h j", j=64)[:],
                                 axis=mybir.AxisListType.X)
            ri = singles.tile([64, 8], f32, tag="ri")
            nc.vector.reciprocal(ri[:], se[:])
            # normalized attention (f16) [i, (h, j)]
            an = singles.tile([64, 512], f16, tag="an")
            for h in range(8):
                nc.vector.tensor_scalar_mul(an[:, 64 * h:64 * (h + 1)],
                                            ex[:, 64 * h:64 * (h + 1)],
                                            ri[:, h:h + 1])
            # transpose attention per head -> amT [j, (h, i)]
            pat = psm.tile([64, 512], f16, tag="m")
            for h in range(8):
                nc.tensor.transpose(pat[:, 64 * h:64 * (h + 1)],
                                    an[:, 64 * h:64 * (h + 1)], ident[:])
            amT = singles.tile([64, 512], f16, tag="amT")
            nc.vector.tensor_copy(amT[:], pat[:])

            # ---- attn @ V -> O [i, (h, jm, d)], then per-jm transposes ----
            pO = psm.tile([64, 512], f32, tag="m")
            for h in range(8):
                nc.tensor.matmul(pO[:, 64 * h:64 * (h + 1)],
                                 amT[:, 64 * h:64 * (h + 1)],
                                 V_sb[:, 64 * h:64 * (h + 1)],
                                 start=True, stop=True)
            O_sb = singles.tile([64, 512], f16, tag="Osb")  # [i, (jm, h, d)]
            nc.vector.tensor_copy(
                O_sb.rearrange("i (j h d) -> i j h d", h=8, d=8)[:],
                pO.rearrange("i (h j d) -> i j h d", j=8, d=8)[:])
            # transpose [s, (h,d)] slice per jm -> [ (h,d), s ]; XOR | XOI
            pXO = psm.tile([64, 512], f16, tag="m")
            for m in range(4):
                nc.tensor.transpose(pXO[:, 64 * m:64 * (m + 1)],
                                    O_sb[:, 128 * m:128 * m + 64], ident[:])
                nc.tensor.transpose(pXO[:, 256 + 64 * m:256 + 64 * (m + 1)],
                                    O_sb[:, 128 * m + 64:128 * (m + 1)], ident[:])
            XO = singles.tile([64, 512], f16, tag="XO")
            nc.vector.tensor_copy(XO[:], pXO[:])
            XOR = XO[:, 0:256].rearrange("c (m s) -> c m s", s=64)
            XOI = XO[:, 256:512].rearrange("c (m s) -> c m s", s=64)

            # ---- w_out complex mixing -> F [c_out, (jm, s)] ----
            wo3 = wor.rearrange("c (o m) -> c o m", m=4)
            woi3 = woi.rearrange("c (o m) -> c o m", m=4)
            woin3 = woin.rearrange("c (o m) -> c o m", m=4)
            pf = psm.tile([64, 512], f32, tag="m")
            for m in range(4):
                rR = XOR[:, m, :]
                rI = XOI[:, m, :]
                blk = pf[:, 64 * (2 * m):64 * (2 * m) + 64]
                nc.tensor.matmul(blk, wo3[:, :, m], rR, start=True, stop=False)
                nc.tensor.matmul(blk, woin3[:, :, m], rI, start=False, stop=True)
                blk = pf[:, 64 * (2 * m + 1):64 * (2 * m + 1) + 64]
                nc.tensor.matmul(blk, woi3[:, :, m], rR, start=True, stop=False)
                nc.tensor.matmul(blk, wo3[:, :, m], rI, start=False, stop=True)
            F_sb = singles.tile([64, 512], f16, tag="Fsb")
            nc.vector.tensor_copy(F_sb[:], pf[:])

            # ---- U_fT [8, (s, c)] via 64 cheap f16 transposes ----
            f3 = F_sb.rearrange("c (j s) -> c j s", s=64)
            U_fT = singles.tile([8, 4096], f16, tag="UfT")
            for g8 in range(8):
                pu = psm.tile([8, 512], f16, tag="m")
                for u in range(8):
                    s = 8 * g8 + u
                    nc.tensor.transpose(pu[:, 64 * u:64 * (u + 1)],
                                        f3[:, :, s], ident[:])
                if g8 % 2 == 0:
                    nc.vector.tensor_copy(U_fT[:, 512 * g8:512 * (g8 + 1)], pu[:])
                else:
                    nc.scalar.copy(U_fT[:, 512 * g8:512 * (g8 + 1)], pu[:])

            if debug:
                nc.sync.dma_start(out=dbg_out["dXsj"][:], in_=Xsj[:])
                nc.sync.dma_start(out=dbg_out["dQ"][:], in_=Q_sb[:])
                nc.sync.dma_start(out=dbg_out["dK"][:], in_=K_sb[:])
                nc.sync.dma_start(out=dbg_out["dV"][:], in_=V_sb[:])
                nc.scalar.dma_start(out=dbg_out["dan"][:], in_=an[:])
                nc.scalar.dma_start(out=dbg_out["dU"][:], in_=U_fT[:])
                dbias16 = singles.tile([64, 512], f16, tag="dbias16")
                nc.vector.tensor_copy(dbias16[:], bias_jh[:])
                nc.scalar.dma_start(out=dbg_out["dbias"][:], in_=dbias16[:])

            # ---- stage 7: y = U_fT^T @ Bas, streamed in 32 chunks ----
            cp_engines = [nc.vector, nc.scalar]
            for t in range(32):
                lh = U_fT[:, 128 * t:128 * (t + 1)]
                py0 = ps7.tile([128, 512], f32, tag="y")
                py1 = ps7.tile([128, 512], f32, tag="y")
                nc.tensor.matmul(py0[:], lh, bas[:, :512], start=True, stop=True)
                nc.tensor.matmul(py1[:], lh, bas[:, 512:], start=True, stop=True)
                y_sb = y_pool.tile([128, 1024], f16, tag="y_sb")
                for half, py in ((0, py0), (1, py1)):
                    ce = cp_engines[(t + half) % 2]
                    dstv = y_sb[:, 512 * half:512 * (half + 1)]
                    if ce is nc.scalar:
                        ce.copy(dstv, py[:])
                    else:
                        ce.tensor_copy(dstv, py[:])
                nc.sync.dma_start(
                    out=y_out[128 * t:128 * (t + 1), :], in_=y_sb[:])
    nc.finalize()
    return nc


_NC_CACHE = {}


def kernel(**inputs) -> np.ndarray:
    from concourse.bass_utils import run_bass_kernel_spmd

    seq = np.asarray(inputs["seq"], dtype=np.float32)
    assert seq.shape == (B, S, C, H, W)

    if "nc" not in _NC_CACHE:
        _NC_CACHE["nc"] = _build()
    nc = _NC_CACHE["nc"]

    e8pk, main, relTpack, bas16 = _pack_weights(inputs)
    common = {"e8pack": e8pk, "mainpack": main, "relTpack": relTpack, "bas": bas16}
    seq16 = seq.reshape(B, 4096, 1024).astype(np.float16)
    in_maps = []
    for b in range(NCORES):
        m = dict(common)
        m["xt"] = np.ascontiguousarray(seq16[b].T)
        in_maps.append(m)

    res = run_bass_kernel_spmd(nc, in_maps, list(range(NCORES)))
    out = np.stack([res.results[b]["y"].astype(np.float32).reshape(S, C, H, W)
                    for b in range(NCORES)])
    return out


# revision 22
# speedup vs baseline: 1.7725x; 1.0195x over previous
"""Trainium2 Bass kernel for nn_FuncSelfAttention (spectral self-attention).

v3 design (cost-model driven):
  - Host casts seq to f16 AND pre-transposes it: x^T [1024 hw, 4096 (s,c)]
    per core (data-parallel over batch, 1 batch element per core). Device
    does 8 plain 1MB DMA loads (no transpose premium, no queue clog).
  - All weights/constants packed host-side into 3 DMAs (ring depth per
    HWDGE queue is 2, so many small loads would serialize at ~1 per big
    transfer otherwise).
  - Projection to 2x2 Fourier modes: 512 tiny matmuls (out [64c, 8jm] per s,
    accumulated over the 8 hw-chunks) into ONE psum bank -> Xall [c,(s,jm)].
  - QKV complex mixing, batched-head attention (one exp/reduce/recip pass),
    attn@V per head, per-jm transposes, w_out mixing, U_fT via 64 f16
    transposes; stage 7 streams y = U_fT^T @ Bas in 32 chunks of [128,1024]:
    PE matmul (f32 psum) -> DVE/Act half-copies (cast f16) -> DMA out on SP.
"""
import numpy as np

B, S, C, H, W = 8, 64, 64, 32, 32
NH, HD = 8, 8
HW = H * W
NCORES = 8
MODES4 = [(0, 0), (0, 1), (1, 0), (1, 1)]

MAIN_COLS = 4106


def _constants():
    hh, ww = np.meshgrid(np.arange(H), np.arange(W), indexing="ij")
    phi, psi = 2 * np.pi / H, 2 * np.pi / W
    E8 = np.zeros((HW, 8))
    Bas = np.zeros((8, HW))
    for mi, (kx, ky) in enumerate(MODES4):
        th = phi * kx * hh + psi * ky * ww
        E8[:, 2 * mi] = np.cos(th).ravel()
        E8[:, 2 * mi + 1] = -np.sin(th).ravel()
        mult = 1.0 if ky == 0 else 2.0
        Bas[2 * mi] = mult / HW * np.cos(th).ravel()
        Bas[2 * mi + 1] = -mult / HW * np.sin(th).ravel()
    g = (Bas @ Bas.T).diagonal().copy()      # attention Gram diag (per jm)
    t8d = (Bas @ E8).diagonal().copy()       # coeff->mode map (diagonal)

    e8c = np.zeros((128, 64), np.float32)    # hw-chunk k of E8 at cols [8k,8k+8)
    for k in range(8):
        e8c[:, 8 * k:8 * k + 8] = E8[128 * k:128 * (k + 1)]

    gx, gy = np.meshgrid(np.arange(8), np.arange(8), indexing="ij")
    coords = np.stack([gx.ravel(), gy.ravel()], -1).astype(np.float32)
    rel = coords[:, None, :] - coords[None, :, :]
    rel = np.sign(rel) * np.log2(1.0 + np.abs(rel))          # [64, 64, 2]
    relT = np.ascontiguousarray(rel.reshape(4096, 2).T).astype(np.float32)

    scale = np.float32(1.0 / HW) / np.float32(np.sqrt(HD))
    gcol = np.zeros((64, 1), np.float32)     # rows (jm, d): p = jm*8+d
    for p in range(64):
        gcol[p, 0] = g[p // 8] * scale
    t8pat = np.zeros((64, 512), np.float32)  # over (h, jm, d): col = h*64+jm*8+d
    for col in range(512):
        t8pat[:, col] = t8d[(col // 8) % 8]
    return e8c, Bas.astype(np.float32), relT, gcol, t8pat


def _pack_weights(inputs):
    """mainpack [128, MAIN_COLS] f16, relTpack [2, 4160] f16, bas [8,1024] f16."""
    e8c, bas, relT, gcol, t8pat = _constants()
    wqr = np.asarray(inputs["w_qkv_r"], np.float32).reshape(64, 768)
    wqi = np.asarray(inputs["w_qkv_i"], np.float32).reshape(64, 768)
    wor = np.asarray(inputs["w_out_r"], np.float32).reshape(64, 256)
    woi = np.asarray(inputs["w_out_i"], np.float32).reshape(64, 256)
    cw1 = np.asarray(inputs["cpb_w1"], np.float32)
    cb1 = np.asarray(inputs["cpb_b1"], np.float32).reshape(64, 1)
    cw2 = np.asarray(inputs["cpb_w2"], np.float32)

    main = np.zeros((64, MAIN_COLS), np.float16)
    main[:, 0:768] = wqr
    main[:, 768:1536] = wqi
    main[:, 1536:2304] = -wqi
    main[:, 2304:2560] = wor
    main[:, 2560:2816] = woi
    main[:, 2816:3072] = -woi
    main[:, 3072:3080] = cw2
    main[:, 3080:3081] = gcol
    main[:, 3081:3082] = cb1
    main[:, 3082:3594] = t8pat[:, :512]
    gpat = np.zeros((64, 512), np.float32)
    for p in range(64):
        gpat[p, :] = gcol[p, 0]
    main[:, 3594:4106] = gpat

    relTpack = np.zeros((2, 4160), np.float16)
    relTpack[:, 0:4096] = relT
    relTpack[:, 4096:4160] = cw1
    return e8c.astype(np.float16), main, relTpack, bas.astype(np.float16)


def _build(debug=False):
    import concourse.bass as bass
    import concourse.mybir as mybir
    import concourse.tile as tile
    from concourse import bacc
    from concourse.masks import make_identity

    f32 = mybir.dt.float32
    f16 = mybir.dt.float16
    Exp = mybir.ActivationFunctionType.Exp
    Relu = mybir.ActivationFunctionType.Relu

    nc = bacc.Bacc("TRN2", target_bir_lowering=False, debug=False,
                   dynamic_dma_scratch_size=49152)
    x_in = nc.dram_tensor("xt", [1024, 4096], f16, kind="ExternalInput")
    e8_in = nc.dram_tensor("e8pack", [128, 64], f16, kind="ExternalInput")
    main_in = nc.dram_tensor("mainpack", [64, MAIN_COLS], f16,
                             kind="ExternalInput")
    rel_in = nc.dram_tensor("relTpack", [2, 4160], f16, kind="ExternalInput")
    bas_in = nc.dram_tensor("bas", [8, 1024], f16, kind="ExternalInput")
    y_out = nc.dram_tensor("y", [4096, 1024], f16, kind="ExternalOutput")
    dbg_out = {}
    if debug:
        for nm in ("dXsj", "dQ", "dK", "dV", "dbias", "dan", "dU"):
            shp = [8, 4096] if nm == "dU" else [64, 512]
            dbg_out[nm] = nc.dram_tensor(nm, shp, f16, kind="ExternalOutput")

    with tile.TileContext(nc) as tc:
        import contextlib
        ctx = contextlib.ExitStack()
        with ctx:
            singles = ctx.enter_context(tc.tile_pool(name="singles", bufs=1))
            psm = ctx.enter_context(tc.tile_pool(name="psm", bufs=2, space="PSUM"))
            psX = ctx.enter_context(tc.tile_pool(name="psX", bufs=2, space="PSUM"))
            ps7 = ctx.enter_context(tc.tile_pool(name="ps7", bufs=4, space="PSUM"))
            y_pool = ctx.enter_context(tc.tile_pool(name="yp", bufs=6))

            # ---- packed constants: 4 DMAs on scalar; x chunks on sync ----
            relTp = singles.tile([2, 4160], f16, tag="relTp")
            nc.scalar.dma_start(out=relTp[:], in_=rel_in[:])
            e8t = singles.tile([128, 64], f16, tag="e8t")
            nc.scalar.dma_start(out=e8t[:], in_=e8_in[:])
            mainp = singles.tile([64, MAIN_COLS], f16, tag="mainp")
            nc.scalar.dma_start(out=mainp[:], in_=main_in[:])
            bas = singles.tile([8, 1024], f16, tag="bas")
            nc.scalar.dma_start(out=bas[:], in_=bas_in[:])

            xT = []
            for k in range(8):
                t = singles.tile([128, 4096], f16, tag=f"xT{k}")
                nc.sync.dma_start(out=t[:], in_=x_in[128 * k:128 * (k + 1), :])
                xT.append(t)

            wqr = mainp[:, 0:768]
            wqi = mainp[:, 768:1536]
            wqin = mainp[:, 1536:2304]
            wor = mainp[:, 2304:2560]
            woi = mainp[:, 2560:2816]
            woin = mainp[:, 2816:3072]
            cw2 = mainp[:, 3072:3080]
            gpat = mainp[:, 3594:4106]
            cb1 = mainp[:, 3081:3082]
            t8rep = mainp[:, 3082:3594]
            relT = relTp[:, 0:4096]
            cw1 = relTp[:, 4096:4160]

            ident = singles.tile([64, 64], f16, tag="ident")
            make_identity(nc, ident[:])

            # ---- projection (interleaved with CPB, which needs no x) ----
            # PSUM accumulation groups must be contiguous, so each chunk is a
            # single-shot matmul set into a rotating psum tile, accumulated
            # into SBUF f32 (last chunk writes the f16 result directly).
            Xacc = singles.tile([64, 512], f32, tag="Xacc")
            Xsj = singles.tile([64, 512], f16, tag="Xsj")   # [c, (s, jm)]

            def proj_chunk(k):
                pXk = psX.tile([64, 512], f32, tag="X")
                for s in range(64):
                    nc.tensor.matmul(pXk[:, 8 * s:8 * s + 8],
                                     xT[k][:, 64 * s:64 * (s + 1)],
                                     e8t[:, 8 * k:8 * k + 8],
                                     start=True, stop=True)
                if k == 0:
                    nc.vector.tensor_copy(Xacc[:], pXk[:])
                elif k < 7:
                    nc.vector.tensor_add(Xacc[:], pXk[:], Xacc[:])
                else:
                    nc.vector.tensor_add(Xsj[:], pXk[:], Xacc[:])

            proj_chunk(0)
            proj_chunk(1)
            # CPB layer 1: h_relu [e=64, (i,j)=4096] = relu(cw1^T @ relT + b1)
            h_relu = singles.tile([64, 4096], f16, tag="hrelu")
            for n in range(8):
                pc = psm.tile([64, 512], f32, tag="m")
                nc.tensor.matmul(pc[:], cw1[:], relT[:, 512 * n:512 * (n + 1)],
                                 start=True, stop=True)
                nc.scalar.activation(h_relu[:, 512 * n:512 * (n + 1)], pc[:],
                                     Relu, bias=cb1[:])
            proj_chunk(2)
            proj_chunk(3)
            # CPB layer 2: bias [i, (j, h)]: 64 matmuls, one copy out
            h3 = h_relu.rearrange("e (i j) -> e i j", j=64)
            pb = psm.tile([64, 512], f32, tag="m")
            for j in range(64):
                nc.tensor.matmul(pb[:, 8 * j:8 * j + 8], h3[:, :, j], cw2[:],
                                 start=True, stop=True)
            bias_jh = singles.tile([64, 512], f32, tag="bias_jh")  # [i, (j, h)]
            nc.vector.tensor_copy(bias_jh[:], pb[:])
            for k in range(4, 8):
                proj_chunk(k)

            Xjs = singles.tile([64, 512], f16, tag="Xjs")   # [c, (jm, s)]
            nc.vector.tensor_copy(
                Xjs.rearrange("c (j s) -> c j s", s=64)[:],
                Xsj.rearrange("c (s j) -> c j s", j=8)[:])
            xv = Xjs.rearrange("c (j s) -> c j s", s=64)

            # ---- QKV complex mixing -> Q/K/V [s, (h, jm, d)] ----
            wq3 = wqr.rearrange("c (o m) -> c o m", m=4)
            wi3 = wqi.rearrange("c (o m) -> c o m", m=4)
            win3 = wqin.rearrange("c (o m) -> c o m", m=4)
            Q_sb = singles.tile([64, 512], f16, tag="Qsb")
            K_sb = singles.tile([64, 512], f16, tag="Ksb")
            V_sb = singles.tile([64, 512], f16, tag="Vsb")
            for qi, (sb, o0) in enumerate(((Q_sb, 0), (K_sb, 64), (V_sb, 128))):
                pd = psm.tile([64, 512], f32, tag="m")
                for m in range(4):
                    lR = xv[:, 2 * m, :]
                    lI = xv[:, 2 * m + 1, :]
                    wR = wq3[:, o0:o0 + 64, m]
                    wI = wi3[:, o0:o0 + 64, m]
                    wIn = win3[:, o0:o0 + 64, m]
                    blk = pd[:, 64 * (2 * m):64 * (2 * m) + 64]
                    nc.tensor.matmul(blk, lR, wR, start=True, stop=False)
                    nc.tensor.matmul(blk, lI, wIn, start=False, stop=True)
                    blk = pd[:, 64 * (2 * m + 1):64 * (2 * m + 1) + 64]
                    nc.tensor.matmul(blk, lR, wI, start=True, stop=False)
                    nc.tensor.matmul(blk, lI, wR, start=False, stop=True)
                pdv = pd.rearrange("s (j h d) -> s j h d", h=8, d=8)
                sbv = sb.rearrange("s (h j d) -> s j h d", j=8, d=8)
                if qi == 0:
                    nc.vector.tensor_copy(sbv[:], pdv[:])
                elif qi == 1:
                    nc.scalar.copy(sbv[:], pdv[:])
                else:
                    # V scaled by t8 diag (fold of the coeff->mode map);
                    # t8rep pattern is laid out for the (h, jm, d) output order
                    nc.vector.tensor_mul(
                        sbv[:], pdv[:],
                        t8rep.rearrange("s (h j d) -> s j h d", j=8, d=8)[:])

            # ---- attention ----
            QT = singles.tile([64, 512], f16, tag="QT")
            KT = singles.tile([64, 512], f16, tag="KT")
            pqt = psm.tile([64, 512], f16, tag="m")
            for h in range(8):
                nc.tensor.transpose(pqt[:, 64 * h:64 * (h + 1)],
                                    Q_sb[:, 64 * h:64 * (h + 1)], ident[:])
            nc.vector.tensor_mul(QT[:], pqt[:], gpat[:])
            pkt = psm.tile([64, 512], f16, tag="m")
            for h in range(8):
                nc.tensor.transpose(pkt[:, 64 * h:64 * (h + 1)],
                                    K_sb[:, 64 * h:64 * (h + 1)], ident[:])
            nc.scalar.copy(KT[:], pkt[:])

            # scores per head into one psum bank [i, (h, j)]
            pS = psm.tile([64, 512], f32, tag="m")
            for h in range(8):
                nc.tensor.matmul(pS[:, 64 * h:64 * (h + 1)],
                                 QT[:, 64 * h:64 * (h + 1)],
                                 KT[:, 64 * h:64 * (h + 1)],
                                 start=True, stop=True)
            # add bias (strided view of [i,(j,h)] -> [i,(h,j)]) and exp
            sc_sb = singles.tile([64, 512], f32, tag="sc")
            bview = bias_jh.rearrange("i (j h) -> i h j", h=8)
            sview = sc_sb.rearrange("i (h j) -> i h j", j=64)
            nc.vector.tensor_add(sview[:],
                                 pS.rearrange("i (h j) -> i h j", j=64)[:],
                                 bview[:])
            ex = singles.tile([64, 512], f32, tag="ex")
            nc.scalar.activation(ex[:], sc_sb[:], Exp)
            se = singles.tile([64, 8], f32, tag="se")
            nc.vector.reduce_sum(se[:], ex.rearrange("i (h j) -> i h j", j=64)[:],
                                 axis=mybir.AxisListType.X)
            ri = singles.tile([64, 8], f32, tag="ri")
            nc.vector.reciprocal(ri[:], se[:])
            # normalized attention (f16) [i, (h, j)]
            an = singles.tile([64, 512], f16, tag="an")
            nc.vector.tensor_mul(
                an.rearrange("i (h j) -> i h j", j=64)[:],
                ex.rearrange("i (h j) -> i h j", j=64)[:],
                ri[:, :, None].to_broadcast([64, 8, 64]))
            # transpose attention per head -> amT [j, (h, i)]
            pat = psm.tile([64, 512], f16, tag="m")
            for h in range(8):
                nc.tensor.transpose(pat[:, 64 * h:64 * (h + 1)],
                                    an[:, 64 * h:64 * (h + 1)], ident[:])
            amT = singles.tile([64, 512], f16, tag="amT")
            nc.vector.tensor_copy(amT[:], pat[:])

            # ---- attn @ V -> O [i, (h, jm, d)], then per-jm transposes ----
            pO = psm.tile([64, 512], f32, tag="m")
            for h in range(8):
                nc.tensor.matmul(pO[:, 64 * h:64 * (h + 1)],
                                 amT[:, 64 * h:64 * (h + 1)],
                                 V_sb[:, 64 * h:64 * (h + 1)],
                                 start=True, stop=True)
            O_sb = singles.tile([64, 512], f16, tag="Osb")  # [i, (jm, h, d)]
            nc.vector.tensor_copy(
                O_sb.rearrange("i (j h d) -> i j h d", h=8, d=8)[:],
                pO.rearrange("i (h j d) -> i j h d", j=8, d=8)[:])
            # transpose [s, (h,d)] slice per jm -> [ (h,d), s ]; XOR | XOI
            pXO = psm.tile([64, 512], f16, tag="m")
            for m in range(4):
                nc.tensor.transpose(pXO[:, 64 * m:64 * (m + 1)],
                                    O_sb[:, 128 * m:128 * m + 64], ident[:])
                nc.tensor.transpose(pXO[:, 256 + 64 * m:256 + 64 * (m + 1)],
                                    O_sb[:, 128 * m + 64:128 * (m + 1)], ident[:])
            XO = singles.tile([64, 512], f16, tag="XO")
            nc.vector.tensor_copy(XO[:], pXO[:])
            XOR = XO[:, 0:256].rearrange("c (m s) -> c m s", s=64)
            XOI = XO[:, 256:512].rearrange("c (m s) -> c m s", s=64)

            # ---- w_out complex mixing -> F [c_out, (jm, s)] ----
            wo3 = wor.rearrange("c (o m) -> c o m", m=4)
            woi3 = woi.rearrange("c (o m) -> c o m", m=4)
            woin3 = woin.rearrange("c (o m) -> c o m", m=4)
            pf = psm.tile([64, 512], f32, tag="m")
            for m in range(4):
                rR = XOR[:, m, :]
                rI = XOI[:, m, :]
                blk = pf[:, 64 * (2 * m):64 * (2 * m) + 64]
                nc.tensor.matmul(blk, wo3[:, :, m], rR, start=True, stop=False)
                nc.tensor.matmul(blk, woin3[:, :, m], rI, start=False, stop=True)
                blk = pf[:, 64 * (2 * m + 1):64 * (2 * m + 1) + 64]
                nc.tensor.matmul(blk, woi3[:, :, m], rR, start=True, stop=False)
                nc.tensor.matmul(blk, wo3[:, :, m], rI, start=False, stop=True)
            F_sb = singles.tile([64, 512], f16, tag="Fsb")
            nc.vector.tensor_copy(F_sb[:], pf[:])

            # ---- U_fT [8, (s, c)] via 64 cheap f16 transposes ----
            f3 = F_sb.rearrange("c (j s) -> c j s", s=64)
            U_fT = []
            for g in range(8):
                uft_g = singles.tile([8, 512], f16, tag=f"UfT{g}")
                U_fT.append(uft_g)
            for g8 in range(8):
                pu = psm.tile([8, 512], f16, tag="m")
                for u in range(8):
                    s = 8 * g8 + u
                    nc.tensor.transpose(pu[:, 64 * u:64 * (u + 1)],
                                        f3[:, :, s], ident[:])
                if g8 % 2 == 0:
                    nc.vector.tensor_copy(U_fT[g8][:], pu[:])
                else:
                    nc.scalar.copy(U_fT[g8][:], pu[:])

            if debug:
                nc.sync.dma_start(out=dbg_out["dXsj"][:], in_=Xsj[:])
                nc.sync.dma_start(out=dbg_out["dQ"][:], in_=Q_sb[:])
                nc.sync.dma_start(out=dbg_out["dK"][:], in_=K_sb[:])
                nc.sync.dma_start(out=dbg_out["dV"][:], in_=V_sb[:])
                nc.scalar.dma_start(out=dbg_out["dan"][:], in_=an[:])
                dbias16 = singles.tile([64, 512], f16, tag="dbias16")
                nc.vector.tensor_copy(dbias16[:], bias_jh[:])
                nc.scalar.dma_start(out=dbg_out["dbias"][:], in_=dbias16[:])

            # ---- stage 7: y = U_fT^T @ Bas, streamed in 32 chunks ----
            cp_engines = [nc.vector, nc.scalar]
            for t in range(32):
                lh = U_fT[t // 4][:, 128 * (t % 4):128 * (t % 4 + 1)]
                py0 = ps7.tile([128, 512], f32, tag="y")
                py1 = ps7.tile([128, 512], f32, tag="y")
                nc.tensor.matmul(py0[:], lh, bas[:, :512], start=True, stop=True)
                nc.tensor.matmul(py1[:], lh, bas[:, 512:], start=True, stop=True)
                y_sb = y_pool.tile([128, 1024], f16, tag="y_sb")
                for half, py in ((0, py0), (1, py1)):
                    ce = cp_engines[(t + half) % 2]
                    dstv = y_sb[:, 512 * half:512 * (half + 1)]
                    if ce is nc.scalar:
                        ce.copy(dstv, py[:])
                    else:
                        ce.tensor_copy(dstv, py[:])
                (nc.sync if t % 2 == 0 else nc.scalar).dma_start(
                    out=y_out[128 * t:128 * (t + 1), :], in_=y_sb[:])
    nc.finalize()
    return nc


_NC_CACHE = {}


def kernel(**inputs) -> np.ndarray:
    from concourse.bass_utils import run_bass_kernel_spmd

    seq = np.asarray(inputs["seq"], dtype=np.float32)
    assert seq.shape == (B, S, C, H, W)

    if "nc" not in _NC_CACHE:
        _NC_CACHE["nc"] = _build()
    nc = _NC_CACHE["nc"]

    e8pk, main, relTpack, bas16 = _pack_weights(inputs)
    common = {"e8pack": e8pk, "mainpack": main, "relTpack": relTpack, "bas": bas16}
    seq16 = seq.reshape(B, 4096, 1024).astype(np.float16)
    in_maps = []
    for b in range(NCORES):
        m = dict(common)
        m["xt"] = np.ascontiguousarray(seq16[b].T)
        in_maps.append(m)

    res = run_bass_kernel_spmd(nc, in_maps, list(range(NCORES)))
    out = np.stack([res.results[b]["y"].astype(np.float32).reshape(S, C, H, W)
                    for b in range(NCORES)])
    return out


# revision 25
# speedup vs baseline: 1.8686x; 1.0543x over previous
"""Trainium2 Bass kernel for nn_FuncSelfAttention (spectral self-attention).

v3 design (cost-model driven):
  - Host casts seq to f16 AND pre-transposes it: x^T [1024 hw, 4096 (s,c)]
    per core (data-parallel over batch, 1 batch element per core). Device
    does 8 plain 1MB DMA loads (no transpose premium, no queue clog).
  - All weights/constants packed host-side into 3 DMAs (ring depth per
    HWDGE queue is 2, so many small loads would serialize at ~1 per big
    transfer otherwise).
  - Projection to 2x2 Fourier modes: 512 tiny matmuls (out [64c, 8jm] per s,
    accumulated over the 8 hw-chunks) into ONE psum bank -> Xall [c,(s,jm)].
  - QKV complex mixing, batched-head attention (one exp/reduce/recip pass),
    attn@V per head, per-jm transposes, w_out mixing, U_fT via 64 f16
    transposes; stage 7 streams y = U_fT^T @ Bas in 32 chunks of [128,1024]:
    PE matmul (f32 psum) -> DVE/Act half-copies (cast f16) -> DMA out on SP.
"""
import numpy as np

B, S, C, H, W = 8, 64, 64, 32, 32
NH, HD = 8, 8
HW = H * W
NCORES = 8
MODES4 = [(0, 0), (0, 1), (1, 0), (1, 1)]

MAIN_COLS = 3072
SMALL_COLS = 1033


def _constants():
    hh, ww = np.meshgrid(np.arange(H), np.arange(W), indexing="ij")
    phi, psi = 2 * np.pi / H, 2 * np.pi / W
    E8 = np.zeros((HW, 8))
    Bas = np.zeros((8, HW))
    for mi, (kx, ky) in enumerate(MODES4):
        th = phi * kx * hh + psi * ky * ww
        E8[:, 2 * mi] = np.cos(th).ravel()
        E8[:, 2 * mi + 1] = -np.sin(th).ravel()
        mult = 1.0 if ky == 0 else 2.0
        Bas[2 * mi] = mult / HW * np.cos(th).ravel()
        Bas[2 * mi + 1] = -mult / HW * np.sin(th).ravel()
    g = (Bas @ Bas.T).diagonal().copy()      # attention Gram diag (per jm)
    t8d = (Bas @ E8).diagonal().copy()       # coeff->mode map (diagonal)

    e8c = np.zeros((128, 64), np.float32)    # hw-chunk k of E8 at cols [8k,8k+8)
    for k in range(8):
        e8c[:, 8 * k:8 * k + 8] = E8[128 * k:128 * (k + 1)]

    gx, gy = np.meshgrid(np.arange(8), np.arange(8), indexing="ij")
    coords = np.stack([gx.ravel(), gy.ravel()], -1).astype(np.float32)
    rel = coords[:, None, :] - coords[None, :, :]
    rel = np.sign(rel) * np.log2(1.0 + np.abs(rel))          # [64, 64, 2]
    relT = np.ascontiguousarray(rel.reshape(4096, 2).T).astype(np.float32)

    scale = np.float32(1.0 / HW) / np.float32(np.sqrt(HD))
    gcol = np.zeros((64, 1), np.float32)     # rows (jm, d): p = jm*8+d
    for p in range(64):
        gcol[p, 0] = g[p // 8] * scale
    t8pat = np.zeros((64, 512), np.float32)  # over (h, jm, d): col = h*64+jm*8+d
    for col in range(512):
        t8pat[:, col] = t8d[(col // 8) % 8]
    return e8c, Bas.astype(np.float32), relT, gcol, t8pat


def _pack_weights(inputs):
    """mainpack [128, MAIN_COLS] f16, relTpack [2, 4160] f16, bas [8,1024] f16."""
    e8c, bas, relT, gcol, t8pat = _constants()
    wqr = np.asarray(inputs["w_qkv_r"], np.float32).reshape(64, 768)
    wqi = np.asarray(inputs["w_qkv_i"], np.float32).reshape(64, 768)
    wor = np.asarray(inputs["w_out_r"], np.float32).reshape(64, 256)
    woi = np.asarray(inputs["w_out_i"], np.float32).reshape(64, 256)
    cw1 = np.asarray(inputs["cpb_w1"], np.float32)
    cb1 = np.asarray(inputs["cpb_b1"], np.float32).reshape(64, 1)
    cw2 = np.asarray(inputs["cpb_w2"], np.float32)

    main = np.zeros((64, MAIN_COLS), np.float16)
    main[:, 0:768] = wqr
    main[:, 768:1536] = wqi
    main[:, 1536:2304] = -wqi
    main[:, 2304:2560] = wor
    main[:, 2560:2816] = woi
    main[:, 2816:3072] = -woi

    small = np.zeros((64, SMALL_COLS), np.float16)
    small[:, 0:8] = cw2
    small[:, 8:9] = cb1
    small[:, 9:521] = t8pat[:, :512]
    gpat = np.zeros((64, 512), np.float32)
    for p in range(64):
        gpat[p, :] = gcol[p, 0]
    small[:, 521:1033] = gpat

    relTpack = np.zeros((2, 4160), np.float16)
    relTpack[:, 0:4096] = relT
    relTpack[:, 4096:4160] = cw1
    return (e8c.astype(np.float16), main, small, relTpack,
            bas.astype(np.float16))


def _build(debug=False):
    import concourse.bass as bass
    import concourse.mybir as mybir
    import concourse.tile as tile
    from concourse import bacc
    from concourse.masks import make_identity

    f32 = mybir.dt.float32
    f16 = mybir.dt.float16
    Exp = mybir.ActivationFunctionType.Exp
    Relu = mybir.ActivationFunctionType.Relu

    nc = bacc.Bacc("TRN2", target_bir_lowering=False, debug=False,
                   dynamic_dma_scratch_size=49152)
    x_in = nc.dram_tensor("xt", [1024, 4096], f16, kind="ExternalInput")
    e8_in = nc.dram_tensor("e8pack", [128, 64], f16, kind="ExternalInput")
    main_in = nc.dram_tensor("mainpack", [64, MAIN_COLS], f16,
                             kind="ExternalInput")
    small_in = nc.dram_tensor("smallpack", [64, SMALL_COLS], f16,
                              kind="ExternalInput")
    rel_in = nc.dram_tensor("relTpack", [2, 4160], f16, kind="ExternalInput")
    bas_in = nc.dram_tensor("bas", [8, 1024], f16, kind="ExternalInput")
    y_out = nc.dram_tensor("y", [4096, 1024], f16, kind="ExternalOutput")
    dbg_out = {}
    if debug:
        for nm in ("dXsj", "dQ", "dK", "dV", "dbias", "dan", "dU"):
            shp = [8, 4096] if nm == "dU" else [64, 512]
            dbg_out[nm] = nc.dram_tensor(nm, shp, f16, kind="ExternalOutput")

    with tile.TileContext(nc) as tc:
        import contextlib
        ctx = contextlib.ExitStack()
        with ctx:
            singles = ctx.enter_context(tc.tile_pool(name="singles", bufs=1))
            psm = ctx.enter_context(tc.tile_pool(name="psm", bufs=2, space="PSUM"))
            psX = ctx.enter_context(tc.tile_pool(name="psX", bufs=1, space="PSUM"))
            ps7 = ctx.enter_context(tc.tile_pool(name="ps7", bufs=5, space="PSUM"))
            y_pool = ctx.enter_context(tc.tile_pool(name="yp", bufs=8))

            # ---- packed constants: 4 DMAs on scalar; x chunks on sync ----
            relTp = singles.tile([2, 4160], f16, tag="relTp")
            nc.scalar.dma_start(out=relTp[:], in_=rel_in[:])
            e8t = singles.tile([128, 64], f16, tag="e8t")
            nc.scalar.dma_start(out=e8t[:], in_=e8_in[:])
            smallp = singles.tile([64, SMALL_COLS], f16, tag="smallp")
            nc.scalar.dma_start(out=smallp[:], in_=small_in[:])

            xT = []
            for k in range(8):
                t = singles.tile([128, 4096], f16, tag=f"xT{k}")
                nc.sync.dma_start(out=t[:], in_=x_in[128 * k:128 * (k + 1), :])
                xT.append(t)

            mainp = singles.tile([64, MAIN_COLS], f16, tag="mainp")
            nc.sync.dma_start(out=mainp[:], in_=main_in[:])
            bas = singles.tile([8, 1024], f16, tag="bas")
            nc.sync.dma_start(out=bas[:], in_=bas_in[:])

            wqr = mainp[:, 0:768]
            wqi = mainp[:, 768:1536]
            wqin = mainp[:, 1536:2304]
            wor = mainp[:, 2304:2560]
            woi = mainp[:, 2560:2816]
            woin = mainp[:, 2816:3072]
            cw2 = smallp[:, 0:8]
            cb1 = smallp[:, 8:9]
            t8rep = smallp[:, 9:521]
            gpat = smallp[:, 521:1033]
            relT = relTp[:, 0:4096]
            cw1 = relTp[:, 4096:4160]

            ident = singles.tile([64, 64], f16, tag="ident")
            make_identity(nc, ident[:])

            # ---- projection (interleaved with CPB, which needs no x) ----
            # PSUM accumulation groups must be contiguous, so each chunk is a
            # single-shot matmul set into a rotating psum tile, accumulated
            # into SBUF f32 (last chunk writes the f16 result directly).
            Xacc = singles.tile([64, 512], f32, tag="Xacc")
            Xsj = singles.tile([64, 512], f16, tag="Xsj")   # [c, (s, jm)]

            def proj_chunk(k):
                pXk = psX.tile([64, 512], f32, tag="X")
                for s in range(64):
                    nc.tensor.matmul(pXk[:, 8 * s:8 * s + 8],
                                     xT[k][:, 64 * s:64 * (s + 1)],
                                     e8t[:, 8 * k:8 * k + 8],
                                     start=True, stop=True)
                if k == 0:
                    nc.vector.tensor_copy(Xacc[:], pXk[:])
                elif k < 7:
                    nc.vector.tensor_add(Xacc[:], pXk[:], Xacc[:])
                else:
                    nc.vector.tensor_add(Xsj[:], pXk[:], Xacc[:])

            proj_chunk(0)
            proj_chunk(1)
            # CPB layer 1: h_relu [e=64, (i,j)=4096] = relu(cw1^T @ relT + b1)
            h_relu = singles.tile([64, 4096], f16, tag="hrelu")
            for n in range(8):
                pc = psm.tile([64, 512], f32, tag="m")
                nc.tensor.matmul(pc[:], cw1[:], relT[:, 512 * n:512 * (n + 1)],
                                 start=True, stop=True)
                nc.scalar.activation(h_relu[:, 512 * n:512 * (n + 1)], pc[:],
                                     Relu, bias=cb1[:])
            proj_chunk(2)
            proj_chunk(3)
            # CPB layer 2: bias [i, (j, h)]: 64 matmuls, one copy out
            h3 = h_relu.rearrange("e (i j) -> e i j", j=64)
            pb = psm.tile([64, 512], f32, tag="m")
            for j in range(64):
                nc.tensor.matmul(pb[:, 8 * j:8 * j + 8], h3[:, :, j], cw2[:],
                                 start=True, stop=True)
            bias_jh = singles.tile([64, 512], f32, tag="bias_jh")  # [i, (j, h)]
            nc.vector.tensor_copy(bias_jh[:], pb[:])
            for k in range(4, 8):
                proj_chunk(k)

            xv = Xsj.rearrange("c (s j) -> c j s", j=8)

            # ---- QKV complex mixing -> Q/K/V [s, (h, jm, d)] ----
            wq3 = wqr.rearrange("c (o m) -> c o m", m=4)
            wi3 = wqi.rearrange("c (o m) -> c o m", m=4)
            win3 = wqin.rearrange("c (o m) -> c o m", m=4)
            Q_sb = singles.tile([64, 512], f16, tag="Qsb")
            K_sb = singles.tile([64, 512], f16, tag="Ksb")
            V_sb = singles.tile([64, 512], f16, tag="Vsb")
            for qi, (sb, o0) in enumerate(((Q_sb, 0), (K_sb, 64), (V_sb, 128))):
                pd = psm.tile([64, 512], f32, tag="m")
                for m in range(4):
                    lR = xv[:, 2 * m, :]
                    lI = xv[:, 2 * m + 1, :]
                    wR = wq3[:, o0:o0 + 64, m]
                    wI = wi3[:, o0:o0 + 64, m]
                    wIn = win3[:, o0:o0 + 64, m]
                    blk = pd[:, 64 * (2 * m):64 * (2 * m) + 64]
                    nc.tensor.matmul(blk, lR, wR, start=True, stop=False)
                    nc.tensor.matmul(blk, lI, wIn, start=False, stop=True)
                    blk = pd[:, 64 * (2 * m + 1):64 * (2 * m + 1) + 64]
                    nc.tensor.matmul(blk, lR, wI, start=True, stop=False)
                    nc.tensor.matmul(blk, lI, wR, start=False, stop=True)
                pdv = pd.rearrange("s (j h d) -> s j h d", h=8, d=8)
                sbv = sb.rearrange("s (h j d) -> s j h d", j=8, d=8)
                if qi == 0:
                    nc.vector.tensor_copy(sbv[:], pdv[:])
                elif qi == 1:
                    nc.scalar.copy(sbv[:], pdv[:])
                else:
                    # V scaled by t8 diag (fold of the coeff->mode map);
                    # t8rep pattern is laid out for the (h, jm, d) output order
                    nc.vector.tensor_mul(
                        sbv[:], pdv[:],
                        t8rep.rearrange("s (h j d) -> s j h d", j=8, d=8)[:])

            # ---- attention ----
            QT = singles.tile([64, 512], f16, tag="QT")
            KT = singles.tile([64, 512], f16, tag="KT")
            pqt = psm.tile([64, 512], f16, tag="m")
            for h in range(8):
                nc.tensor.transpose(pqt[:, 64 * h:64 * (h + 1)],
                                    Q_sb[:, 64 * h:64 * (h + 1)], ident[:])
            nc.vector.tensor_mul(QT[:], pqt[:], gpat[:])
            pkt = psm.tile([64, 512], f16, tag="m")
            for h in range(8):
                nc.tensor.transpose(pkt[:, 64 * h:64 * (h + 1)],
                                    K_sb[:, 64 * h:64 * (h + 1)], ident[:])
            nc.scalar.copy(KT[:], pkt[:])

            # scores per head into one psum bank [i, (h, j)]
            pS = psm.tile([64, 512], f32, tag="m")
            for h in range(8):
                nc.tensor.matmul(pS[:, 64 * h:64 * (h + 1)],
                                 QT[:, 64 * h:64 * (h + 1)],
                                 KT[:, 64 * h:64 * (h + 1)],
                                 start=True, stop=True)
            # add bias (strided view of [i,(j,h)] -> [i,(h,j)]) and exp
            sc_sb = singles.tile([64, 512], f32, tag="sc")
            bview = bias_jh.rearrange("i (j h) -> i h j", h=8)
            sview = sc_sb.rearrange("i (h j) -> i h j", j=64)
            nc.vector.tensor_add(sview[:],
                                 pS.rearrange("i (h j) -> i h j", j=64)[:],
                                 bview[:])
            ex = singles.tile([64, 512], f32, tag="ex")
            nc.scalar.activation(ex[:], sc_sb[:], Exp)
            se = singles.tile([64, 8], f32, tag="se")
            nc.vector.reduce_sum(se[:], ex.rearrange("i (h j) -> i h j", j=64)[:],
                                 axis=mybir.AxisListType.X)
            ri = singles.tile([64, 8], f32, tag="ri")
            nc.vector.reciprocal(ri[:], se[:])
            # normalized attention (f16) [i, (h, j)]
            an = singles.tile([64, 512], f16, tag="an")
            nc.vector.tensor_mul(
                an.rearrange("i (h j) -> i h j", j=64)[:],
                ex.rearrange("i (h j) -> i h j", j=64)[:],
                ri[:, :, None].to_broadcast([64, 8, 64]))
            # transpose attention per head -> amT [j, (h, i)]
            pat = psm.tile([64, 512], f16, tag="m")
            for h in range(8):
                nc.tensor.transpose(pat[:, 64 * h:64 * (h + 1)],
                                    an[:, 64 * h:64 * (h + 1)], ident[:])
            amT = singles.tile([64, 512], f16, tag="amT")
            nc.vector.tensor_copy(amT[:], pat[:])

            # ---- attn @ V -> O [i, (h, jm, d)], then per-jm transposes ----
            pO = psm.tile([64, 512], f32, tag="m")
            for h in range(8):
                nc.tensor.matmul(pO[:, 64 * h:64 * (h + 1)],
                                 amT[:, 64 * h:64 * (h + 1)],
                                 V_sb[:, 64 * h:64 * (h + 1)],
                                 start=True, stop=True)
            O_sb = singles.tile([64, 512], f16, tag="Osb")  # [i, (jm, h, d)]
            nc.vector.tensor_copy(
                O_sb.rearrange("i (j h d) -> i j h d", h=8, d=8)[:],
                pO.rearrange("i (h j d) -> i j h d", j=8, d=8)[:])
            # transpose [s, (h,d)] slice per jm -> [ (h,d), s ]; XOR | XOI
            pXO = psm.tile([64, 512], f16, tag="m")
            for m in range(4):
                nc.tensor.transpose(pXO[:, 64 * m:64 * (m + 1)],
                                    O_sb[:, 128 * m:128 * m + 64], ident[:])
                nc.tensor.transpose(pXO[:, 256 + 64 * m:256 + 64 * (m + 1)],
                                    O_sb[:, 128 * m + 64:128 * (m + 1)], ident[:])
            XO = singles.tile([64, 512], f16, tag="XO")
            nc.vector.tensor_copy(XO[:], pXO[:])
            XOR = XO[:, 0:256].rearrange("c (m s) -> c m s", s=64)
            XOI = XO[:, 256:512].rearrange("c (m s) -> c m s", s=64)

            # ---- w_out complex mixing -> F [c_out, (jm, s)] ----
            wo3 = wor.rearrange("c (o m) -> c o m", m=4)
            woi3 = woi.rearrange("c (o m) -> c o m", m=4)
            woin3 = woin.rearrange("c (o m) -> c o m", m=4)
            pf = psm.tile([64, 512], f32, tag="m")
            for m in range(4):
                rR = XOR[:, m, :]
                rI = XOI[:, m, :]
                blk = pf[:, 64 * (2 * m):64 * (2 * m) + 64]
                nc.tensor.matmul(blk, wo3[:, :, m], rR, start=True, stop=False)
                nc.tensor.matmul(blk, woin3[:, :, m], rI, start=False, stop=True)
                blk = pf[:, 64 * (2 * m + 1):64 * (2 * m + 1) + 64]
                nc.tensor.matmul(blk, woi3[:, :, m], rR, start=True, stop=False)
                nc.tensor.matmul(blk, wo3[:, :, m], rI, start=False, stop=True)
            F_sb = singles.tile([64, 512], f16, tag="Fsb")
            nc.vector.tensor_copy(F_sb[:], pf[:])

            # ---- U_fT [8, (s, c)] via 64 cheap f16 transposes ----
            f3 = F_sb.rearrange("c (j s) -> c j s", s=64)
            U_fT = []
            for g in range(8):
                uft_g = singles.tile([8, 512], f16, tag=f"UfT{g}")
                U_fT.append(uft_g)
            for g8 in range(8):
                pu = psm.tile([8, 512], f16, tag="m")
                for u in range(8):
                    s = 8 * g8 + u
                    nc.tensor.transpose(pu[:, 64 * u:64 * (u + 1)],
                                        f3[:, :, s], ident[:])
                if g8 % 2 == 0:
                    nc.vector.tensor_copy(U_fT[g8][:], pu[:])
                else:
                    nc.scalar.copy(U_fT[g8][:], pu[:])

            if debug:
                nc.sync.dma_start(out=dbg_out["dXsj"][:], in_=Xsj[:])
                nc.sync.dma_start(out=dbg_out["dQ"][:], in_=Q_sb[:])
                nc.sync.dma_start(out=dbg_out["dK"][:], in_=K_sb[:])
                nc.sync.dma_start(out=dbg_out["dV"][:], in_=V_sb[:])
                nc.scalar.dma_start(out=dbg_out["dan"][:], in_=an[:])
                dbias16 = singles.tile([64, 512], f16, tag="dbias16")
                nc.vector.tensor_copy(dbias16[:], bias_jh[:])
                nc.scalar.dma_start(out=dbg_out["dbias"][:], in_=dbias16[:])

            # ---- stage 7: y = U_fT^T @ Bas, streamed in 32 chunks ----
            cp_engines = [nc.vector, nc.scalar]
            for t in range(32):
                lh = U_fT[t // 4][:, 128 * (t % 4):128 * (t % 4 + 1)]
                py0 = ps7.tile([128, 512], f32, tag="y")
                py1 = ps7.tile([128, 512], f32, tag="y")
                nc.tensor.matmul(py0[:], lh, bas[:, :512], start=True, stop=True)
                nc.tensor.matmul(py1[:], lh, bas[:, 512:], start=True, stop=True)
                y_sb = y_pool.tile([128, 1024], f16, tag="y_sb")
                for half, py in ((0, py0), (1, py1)):
                    ce = cp_engines[(t + half) % 2]
                    dstv = y_sb[:, 512 * half:512 * (half + 1)]
                    if ce is nc.scalar:
                        ce.copy(dstv, py[:])
                    else:
                        ce.tensor_copy(dstv, py[:])
                (nc.scalar if t % 3 == 2 else nc.sync).dma_start(
                    out=y_out[128 * t:128 * (t + 1), :], in_=y_sb[:])
    nc.finalize()
    return nc


_NC_CACHE = {}


def kernel(**inputs) -> np.ndarray:
    from concourse.bass_utils import run_bass_kernel_spmd

    seq = np.asarray(inputs["seq"], dtype=np.float32)
    assert seq.shape == (B, S, C, H, W)

    if "nc" not in _NC_CACHE:
        _NC_CACHE["nc"] = _build()
    nc = _NC_CACHE["nc"]

    e8pk, main, small, relTpack, bas16 = _pack_weights(inputs)
    common = {"e8pack": e8pk, "mainpack": main, "smallpack": small,
              "relTpack": relTpack, "bas": bas16}
    seq16 = seq.reshape(B, 4096, 1024).astype(np.float16)
    in_maps = []
    for b in range(NCORES):
        m = dict(common)
        m["xt"] = np.ascontiguousarray(seq16[b].T)
        in_maps.append(m)

    res = run_bass_kernel_spmd(nc, in_maps, list(range(NCORES)))
    out = np.stack([res.results[b]["y"].astype(np.float32).reshape(S, C, H, W)
                    for b in range(NCORES)])
    return out


# revision 26
# speedup vs baseline: 1.8738x; 1.0028x over previous
"""Trainium2 Bass kernel for nn_FuncSelfAttention (spectral self-attention).

v3 design (cost-model driven):
  - Host casts seq to f16 AND pre-transposes it: x^T [1024 hw, 4096 (s,c)]
    per core (data-parallel over batch, 1 batch element per core). Device
    does 8 plain 1MB DMA loads (no transpose premium, no queue clog).
  - All weights/constants packed host-side into 3 DMAs (ring depth per
    HWDGE queue is 2, so many small loads would serialize at ~1 per big
    transfer otherwise).
  - Projection to 2x2 Fourier modes: 512 tiny matmuls (out [64c, 8jm] per s,
    accumulated over the 8 hw-chunks) into ONE psum bank -> Xall [c,(s,jm)].
  - QKV complex mixing, batched-head attention (one exp/reduce/recip pass),
    attn@V per head, per-jm transposes, w_out mixing, U_fT via 64 f16
    transposes; stage 7 streams y = U_fT^T @ Bas in 32 chunks of [128,1024]:
    PE matmul (f32 psum) -> DVE/Act half-copies (cast f16) -> DMA out on SP.
"""
import numpy as np

B, S, C, H, W = 8, 64, 64, 32, 32
NH, HD = 8, 8
HW = H * W
NCORES = 8
MODES4 = [(0, 0), (0, 1), (1, 0), (1, 1)]

MAIN_COLS = 3072
SMALL_COLS = 1033


def _constants():
    hh, ww = np.meshgrid(np.arange(H), np.arange(W), indexing="ij")
    phi, psi = 2 * np.pi / H, 2 * np.pi / W
    E8 = np.zeros((HW, 8))
    Bas = np.zeros((8, HW))
    for mi, (kx, ky) in enumerate(MODES4):
        th = phi * kx * hh + psi * ky * ww
        E8[:, 2 * mi] = np.cos(th).ravel()
        E8[:, 2 * mi + 1] = -np.sin(th).ravel()
        mult = 1.0 if ky == 0 else 2.0
        Bas[2 * mi] = mult / HW * np.cos(th).ravel()
        Bas[2 * mi + 1] = -mult / HW * np.sin(th).ravel()
    g = (Bas @ Bas.T).diagonal().copy()      # attention Gram diag (per jm)
    t8d = (Bas @ E8).diagonal().copy()       # coeff->mode map (diagonal)

    e8c = np.zeros((128, 64), np.float32)    # hw-chunk k of E8 at cols [8k,8k+8)
    for k in range(8):
        e8c[:, 8 * k:8 * k + 8] = E8[128 * k:128 * (k + 1)]

    gx, gy = np.meshgrid(np.arange(8), np.arange(8), indexing="ij")
    coords = np.stack([gx.ravel(), gy.ravel()], -1).astype(np.float32)
    rel = coords[:, None, :] - coords[None, :, :]
    rel = np.sign(rel) * np.log2(1.0 + np.abs(rel))          # [64, 64, 2]
    relT = np.ascontiguousarray(rel.reshape(4096, 2).T).astype(np.float32)

    scale = np.float32(1.0 / HW) / np.float32(np.sqrt(HD))
    gcol = np.zeros((64, 1), np.float32)     # rows (jm, d): p = jm*8+d
    for p in range(64):
        gcol[p, 0] = g[p // 8] * scale
    t8pat = np.zeros((64, 512), np.float32)  # over (h, jm, d): col = h*64+jm*8+d
    for col in range(512):
        t8pat[:, col] = t8d[(col // 8) % 8]
    return e8c, Bas.astype(np.float32), relT, gcol, t8pat


def _pack_weights(inputs):
    """mainpack [128, MAIN_COLS] f16, relTpack [2, 4160] f16, bas [8,1024] f16."""
    e8c, bas, relT, gcol, t8pat = _constants()
    wqr = np.asarray(inputs["w_qkv_r"], np.float32).reshape(64, 768)
    wqi = np.asarray(inputs["w_qkv_i"], np.float32).reshape(64, 768)
    wor = np.asarray(inputs["w_out_r"], np.float32).reshape(64, 256)
    woi = np.asarray(inputs["w_out_i"], np.float32).reshape(64, 256)
    cw1 = np.asarray(inputs["cpb_w1"], np.float32)
    cb1 = np.asarray(inputs["cpb_b1"], np.float32).reshape(64, 1)
    cw2 = np.asarray(inputs["cpb_w2"], np.float32)

    main = np.zeros((64, MAIN_COLS), np.float16)
    main[:, 0:768] = wqr
    main[:, 768:1536] = wqi
    main[:, 1536:2304] = -wqi
    main[:, 2304:2560] = wor
    main[:, 2560:2816] = woi
    main[:, 2816:3072] = -woi

    small = np.zeros((64, SMALL_COLS), np.float16)
    small[:, 0:8] = cw2
    small[:, 8:9] = cb1
    small[:, 9:521] = t8pat[:, :512]
    gpat = np.zeros((64, 512), np.float32)
    for p in range(64):
        gpat[p, :] = gcol[p, 0]
    small[:, 521:1033] = gpat

    relTpack = np.zeros((2, 4160), np.float16)
    relTpack[:, 0:4096] = relT
    relTpack[:, 4096:4160] = cw1
    return (e8c.astype(np.float16), main, small, relTpack,
            bas.astype(np.float16))


def _build(debug=False):
    import concourse.bass as bass
    import concourse.mybir as mybir
    import concourse.tile as tile
    from concourse import bacc
    from concourse.masks import make_identity

    f32 = mybir.dt.float32
    f16 = mybir.dt.float16
    Exp = mybir.ActivationFunctionType.Exp
    Relu = mybir.ActivationFunctionType.Relu

    nc = bacc.Bacc("TRN2", target_bir_lowering=False, debug=False,
                   dynamic_dma_scratch_size=49152)
    x_in = nc.dram_tensor("xt", [1024, 4096], f16, kind="ExternalInput")
    e8_in = nc.dram_tensor("e8pack", [128, 64], f16, kind="ExternalInput")
    main_in = nc.dram_tensor("mainpack", [64, MAIN_COLS], f16,
                             kind="ExternalInput")
    small_in = nc.dram_tensor("smallpack", [64, SMALL_COLS], f16,
                              kind="ExternalInput")
    rel_in = nc.dram_tensor("relTpack", [2, 4160], f16, kind="ExternalInput")
    bas_in = nc.dram_tensor("bas", [8, 1024], f16, kind="ExternalInput")
    y_out = nc.dram_tensor("y", [4096, 1024], f16, kind="ExternalOutput")
    dbg_out = {}
    if debug:
        for nm in ("dXsj", "dQ", "dK", "dV", "dbias", "dan", "dU"):
            shp = [8, 4096] if nm == "dU" else [64, 512]
            dbg_out[nm] = nc.dram_tensor(nm, shp, f16, kind="ExternalOutput")

    with tile.TileContext(nc) as tc:
        import contextlib
        ctx = contextlib.ExitStack()
        with ctx:
            singles = ctx.enter_context(tc.tile_pool(name="singles", bufs=1))
            psm = ctx.enter_context(tc.tile_pool(name="psm", bufs=2, space="PSUM"))
            psX = ctx.enter_context(tc.tile_pool(name="psX", bufs=1, space="PSUM"))
            ps7 = ctx.enter_context(tc.tile_pool(name="ps7", bufs=5, space="PSUM"))
            y_pool = ctx.enter_context(tc.tile_pool(name="yp", bufs=8))

            # ---- packed constants: 4 DMAs on scalar; x chunks on sync ----
            relTp = singles.tile([2, 4160], f16, tag="relTp")
            nc.scalar.dma_start(out=relTp[:], in_=rel_in[:])
            e8t = singles.tile([128, 64], f16, tag="e8t")
            nc.scalar.dma_start(out=e8t[:], in_=e8_in[:])
            smallp = singles.tile([64, SMALL_COLS], f16, tag="smallp")
            nc.scalar.dma_start(out=smallp[:], in_=small_in[:])

            xT = []
            for k in range(8):
                t = singles.tile([128, 4096], f16, tag=f"xT{k}")
                nc.sync.dma_start(out=t[:], in_=x_in[128 * k:128 * (k + 1), :])
                xT.append(t)

            mainp = singles.tile([64, MAIN_COLS], f16, tag="mainp")
            nc.sync.dma_start(out=mainp[:], in_=main_in[:])
            bas = singles.tile([8, 1024], f16, tag="bas")
            nc.sync.dma_start(out=bas[:], in_=bas_in[:])

            wqr = mainp[:, 0:768]
            wqi = mainp[:, 768:1536]
            wqin = mainp[:, 1536:2304]
            wor = mainp[:, 2304:2560]
            woi = mainp[:, 2560:2816]
            woin = mainp[:, 2816:3072]
            cw2 = smallp[:, 0:8]
            cb1 = smallp[:, 8:9]
            t8rep = smallp[:, 9:521]
            gpat = smallp[:, 521:1033]
            relT = relTp[:, 0:4096]
            cw1 = relTp[:, 4096:4160]

            ident = singles.tile([64, 64], f16, tag="ident")
            make_identity(nc, ident[:])

            # ---- projection (interleaved with CPB, which needs no x) ----
            # PSUM accumulation groups must be contiguous, so each chunk is a
            # single-shot matmul set into a rotating psum tile, accumulated
            # into SBUF f32 (last chunk writes the f16 result directly).
            Xacc = singles.tile([64, 512], f32, tag="Xacc")
            Xsj = singles.tile([64, 512], f16, tag="Xsj")   # [c, (s, jm)]

            def proj_chunk(k):
                pXk = psX.tile([64, 512], f32, tag="X")
                for s in range(64):
                    nc.tensor.matmul(pXk[:, 8 * s:8 * s + 8],
                                     xT[k][:, 64 * s:64 * (s + 1)],
                                     e8t[:, 8 * k:8 * k + 8],
                                     start=True, stop=True)
                if k == 0:
                    nc.vector.tensor_copy(Xacc[:], pXk[:])
                elif k < 7:
                    nc.vector.tensor_add(Xacc[:], pXk[:], Xacc[:])
                else:
                    nc.vector.tensor_add(Xsj[:], pXk[:], Xacc[:])

            proj_chunk(0)
            proj_chunk(1)
            # CPB layer 1: h_relu [e=64, (i,j)=4096] = relu(cw1^T @ relT + b1)
            h_relu = singles.tile([64, 4096], f16, tag="hrelu")
            for n in range(8):
                pc = psm.tile([64, 512], f32, tag="m")
                nc.tensor.matmul(pc[:], cw1[:], relT[:, 512 * n:512 * (n + 1)],
                                 start=True, stop=True)
                nc.scalar.activation(h_relu[:, 512 * n:512 * (n + 1)], pc[:],
                                     Relu, bias=cb1[:])
            proj_chunk(2)
            proj_chunk(3)
            # CPB layer 2: bias [i, (j, h)]: 64 matmuls, one copy out
            h3 = h_relu.rearrange("e (i j) -> e i j", j=64)
            pb = psm.tile([64, 512], f32, tag="m")
            for j in range(64):
                nc.tensor.matmul(pb[:, 8 * j:8 * j + 8], h3[:, :, j], cw2[:],
                                 start=True, stop=True)
            bias_jh = singles.tile([64, 512], f32, tag="bias_jh")  # [i, (j, h)]
            nc.vector.tensor_copy(bias_jh[:], pb[:])
            for k in range(4, 8):
                proj_chunk(k)

            xv = Xsj.rearrange("c (s j) -> c j s", j=8)

            # ---- QKV complex mixing -> Q/K/V [s, (h, jm, d)] ----
            wq3 = wqr.rearrange("c (o m) -> c o m", m=4)
            wi3 = wqi.rearrange("c (o m) -> c o m", m=4)
            win3 = wqin.rearrange("c (o m) -> c o m", m=4)
            Q_sb = singles.tile([64, 512], f16, tag="Qsb")
            K_sb = singles.tile([64, 512], f16, tag="Ksb")
            V_sb = singles.tile([64, 512], f16, tag="Vsb")
            for qi, (sb, o0) in enumerate(((Q_sb, 0), (K_sb, 64), (V_sb, 128))):
                pd = psm.tile([64, 512], f32, tag="m")
                for m in range(4):
                    lR = xv[:, 2 * m, :]
                    lI = xv[:, 2 * m + 1, :]
                    wR = wq3[:, o0:o0 + 64, m]
                    wI = wi3[:, o0:o0 + 64, m]
                    wIn = win3[:, o0:o0 + 64, m]
                    blk = pd[:, 64 * (2 * m):64 * (2 * m) + 64]
                    nc.tensor.matmul(blk, lR, wR, start=True, stop=False)
                    nc.tensor.matmul(blk, lI, wIn, start=False, stop=True)
                    blk = pd[:, 64 * (2 * m + 1):64 * (2 * m + 1) + 64]
                    nc.tensor.matmul(blk, lR, wI, start=True, stop=False)
                    nc.tensor.matmul(blk, lI, wR, start=False, stop=True)
                pdv = pd.rearrange("s (j h d) -> s j h d", h=8, d=8)
                sbv = sb.rearrange("s (h j d) -> s j h d", j=8, d=8)
                if qi == 0:
                    nc.vector.tensor_copy(sbv[:], pdv[:])
                elif qi == 1:
                    nc.scalar.copy(sbv[:], pdv[:])
                else:
                    # V scaled by t8 diag (fold of the coeff->mode map);
                    # t8rep pattern is laid out for the (h, jm, d) output order
                    nc.vector.tensor_mul(
                        sbv[:], pdv[:],
                        t8rep.rearrange("s (h j d) -> s j h d", j=8, d=8)[:])

            # ---- attention ----
            QT = singles.tile([64, 512], f16, tag="QT")
            KT = singles.tile([64, 512], f16, tag="KT")
            pqt = psm.tile([64, 512], f16, tag="m")
            for h in range(8):
                nc.tensor.transpose(pqt[:, 64 * h:64 * (h + 1)],
                                    Q_sb[:, 64 * h:64 * (h + 1)], ident[:])
            nc.vector.tensor_mul(QT[:], pqt[:], gpat[:])
            pkt = psm.tile([64, 512], f16, tag="m")
            for h in range(8):
                nc.tensor.transpose(pkt[:, 64 * h:64 * (h + 1)],
                                    K_sb[:, 64 * h:64 * (h + 1)], ident[:])
            nc.scalar.copy(KT[:], pkt[:])

            # scores per head into one psum bank [i, (h, j)]
            pS = psm.tile([64, 512], f32, tag="m")
            for h in range(8):
                nc.tensor.matmul(pS[:, 64 * h:64 * (h + 1)],
                                 QT[:, 64 * h:64 * (h + 1)],
                                 KT[:, 64 * h:64 * (h + 1)],
                                 start=True, stop=True)
            # add bias and exp, pipelined in head-halves across DVE/Act
            sc_sb = singles.tile([64, 512], f32, tag="sc")
            ex = singles.tile([64, 512], f32, tag="ex")
            se = singles.tile([64, 8], f32, tag="se")
            ri = singles.tile([64, 8], f32, tag="ri")
            bview = bias_jh.rearrange("i (j h) -> i h j", h=8)
            sview = sc_sb.rearrange("i (h j) -> i h j", j=64)
            pview = pS.rearrange("i (h j) -> i h j", j=64)
            eview = ex.rearrange("i (h j) -> i h j", j=64)
            for hh in range(2):
                hs = slice(4 * hh, 4 * (hh + 1))
                nc.vector.tensor_add(sview[:, hs], pview[:, hs], bview[:, hs])
                nc.scalar.activation(ex[:, 256 * hh:256 * (hh + 1)],
                                     sc_sb[:, 256 * hh:256 * (hh + 1)], Exp)
                nc.vector.reduce_sum(se[:, hs], eview[:, hs],
                                     axis=mybir.AxisListType.X)
            nc.vector.reciprocal(ri[:], se[:])
            # normalized attention (f16) [i, (h, j)]
            an = singles.tile([64, 512], f16, tag="an")
            nc.vector.tensor_mul(
                an.rearrange("i (h j) -> i h j", j=64)[:],
                eview[:],
                ri[:, :, None].to_broadcast([64, 8, 64]))
            # transpose attention per head -> amT [j, (h, i)]
            pat = psm.tile([64, 512], f16, tag="m")
            for h in range(8):
                nc.tensor.transpose(pat[:, 64 * h:64 * (h + 1)],
                                    an[:, 64 * h:64 * (h + 1)], ident[:])
            amT = singles.tile([64, 512], f16, tag="amT")
            nc.vector.tensor_copy(amT[:], pat[:])

            # ---- attn @ V -> O [i, (h, jm, d)], then per-jm transposes ----
            pO = psm.tile([64, 512], f32, tag="m")
            for h in range(8):
                nc.tensor.matmul(pO[:, 64 * h:64 * (h + 1)],
                                 amT[:, 64 * h:64 * (h + 1)],
                                 V_sb[:, 64 * h:64 * (h + 1)],
                                 start=True, stop=True)
            O_sb = singles.tile([64, 512], f16, tag="Osb")  # [i, (jm, h, d)]
            nc.vector.tensor_copy(
                O_sb.rearrange("i (j h d) -> i j h d", h=8, d=8)[:],
                pO.rearrange("i (h j d) -> i j h d", j=8, d=8)[:])
            # transpose [s, (h,d)] slice per jm -> [ (h,d), s ]; XOR | XOI
            pXO = psm.tile([64, 512], f16, tag="m")
            for m in range(4):
                nc.tensor.transpose(pXO[:, 64 * m:64 * (m + 1)],
                                    O_sb[:, 128 * m:128 * m + 64], ident[:])
                nc.tensor.transpose(pXO[:, 256 + 64 * m:256 + 64 * (m + 1)],
                                    O_sb[:, 128 * m + 64:128 * (m + 1)], ident[:])
            XO = singles.tile([64, 512], f16, tag="XO")
            nc.vector.tensor_copy(XO[:], pXO[:])
            XOR = XO[:, 0:256].rearrange("c (m s) -> c m s", s=64)
            XOI = XO[:, 256:512].rearrange("c (m s) -> c m s", s=64)

            # ---- w_out complex mixing -> F [c_out, (jm, s)] ----
            wo3 = wor.rearrange("c (o m) -> c o m", m=4)
            woi3 = woi.rearrange("c (o m) -> c o m", m=4)
            woin3 = woin.rearrange("c (o m) -> c o m", m=4)
            pf = psm.tile([64, 512], f32, tag="m")
            for m in range(4):
                rR = XOR[:, m, :]
                rI = XOI[:, m, :]
                blk = pf[:, 64 * (2 * m):64 * (2 * m) + 64]
                nc.tensor.matmul(blk, wo3[:, :, m], rR, start=True, stop=False)
                nc.tensor.matmul(blk, woin3[:, :, m], rI, start=False, stop=True)
                blk = pf[:, 64 * (2 * m + 1):64 * (2 * m + 1) + 64]
                nc.tensor.matmul(blk, woi3[:, :, m], rR, start=True, stop=False)
                nc.tensor.matmul(blk, wo3[:, :, m], rI, start=False, stop=True)
            F_sb = singles.tile([64, 512], f16, tag="Fsb")
            nc.vector.tensor_copy(F_sb[:], pf[:])

            # ---- U_fT [8, (s, c)] via 64 cheap f16 transposes ----
            f3 = F_sb.rearrange("c (j s) -> c j s", s=64)
            U_fT = []
            for g in range(8):
                uft_g = singles.tile([8, 512], f16, tag=f"UfT{g}")
                U_fT.append(uft_g)
            for g8 in range(8):
                pu = psm.tile([8, 512], f16, tag="m")
                for u in range(8):
                    s = 8 * g8 + u
                    nc.tensor.transpose(pu[:, 64 * u:64 * (u + 1)],
                                        f3[:, :, s], ident[:])
                if g8 % 2 == 0:
                    nc.vector.tensor_copy(U_fT[g8][:], pu[:])
                else:
                    nc.scalar.copy(U_fT[g8][:], pu[:])

            if debug:
                nc.sync.dma_start(out=dbg_out["dXsj"][:], in_=Xsj[:])
                nc.sync.dma_start(out=dbg_out["dQ"][:], in_=Q_sb[:])
                nc.sync.dma_start(out=dbg_out["dK"][:], in_=K_sb[:])
                nc.sync.dma_start(out=dbg_out["dV"][:], in_=V_sb[:])
                nc.scalar.dma_start(out=dbg_out["dan"][:], in_=an[:])
                dbias16 = singles.tile([64, 512], f16, tag="dbias16")
                nc.vector.tensor_copy(dbias16[:], bias_jh[:])
                nc.scalar.dma_start(out=dbg_out["dbias"][:], in_=dbias16[:])

            # ---- stage 7: y = U_fT^T @ Bas, streamed in 32 chunks ----
            cp_engines = [nc.vector, nc.scalar]
            for t in range(32):
                lh = U_fT[t // 4][:, 128 * (t % 4):128 * (t % 4 + 1)]
                py0 = ps7.tile([128, 512], f32, tag="y")
                py1 = ps7.tile([128, 512], f32, tag="y")
                nc.tensor.matmul(py0[:], lh, bas[:, :512], start=True, stop=True)
                nc.tensor.matmul(py1[:], lh, bas[:, 512:], start=True, stop=True)
                y_sb = y_pool.tile([128, 1024], f16, tag="y_sb")
                for half, py in ((0, py0), (1, py1)):
                    ce = cp_engines[(t + half) % 2]
                    dstv = y_sb[:, 512 * half:512 * (half + 1)]
                    if ce is nc.scalar:
                        ce.copy(dstv, py[:])
                    else:
                        ce.tensor_copy(dstv, py[:])
                (nc.scalar if t % 3 == 2 else nc.sync).dma_start(
                    out=y_out[128 * t:128 * (t + 1), :], in_=y_sb[:])
    nc.finalize()
    return nc


_NC_CACHE = {}


def kernel(**inputs) -> np.ndarray:
    from concourse.bass_utils import run_bass_kernel_spmd

    seq = np.asarray(inputs["seq"], dtype=np.float32)
    assert seq.shape == (B, S, C, H, W)

    if "nc" not in _NC_CACHE:
        _NC_CACHE["nc"] = _build()
    nc = _NC_CACHE["nc"]

    e8pk, main, small, relTpack, bas16 = _pack_weights(inputs)
    common = {"e8pack": e8pk, "mainpack": main, "smallpack": small,
              "relTpack": relTpack, "bas": bas16}
    seq16 = seq.reshape(B, 4096, 1024).astype(np.float16)
    in_maps = []
    for b in range(NCORES):
        m = dict(common)
        m["xt"] = np.ascontiguousarray(seq16[b].T)
        in_maps.append(m)

    res = run_bass_kernel_spmd(nc, in_maps, list(range(NCORES)))
    out = np.stack([res.results[b]["y"].astype(np.float32).reshape(S, C, H, W)
                    for b in range(NCORES)])
    return out


# revision 27
# speedup vs baseline: 1.9062x; 1.0173x over previous
"""Trainium2 Bass kernel for nn_FuncSelfAttention (spectral self-attention).

v3 design (cost-model driven):
  - Host casts seq to f16 AND pre-transposes it: x^T [1024 hw, 4096 (s,c)]
    per core (data-parallel over batch, 1 batch element per core). Device
    does 8 plain 1MB DMA loads (no transpose premium, no queue clog).
  - All weights/constants packed host-side into 3 DMAs (ring depth per
    HWDGE queue is 2, so many small loads would serialize at ~1 per big
    transfer otherwise).
  - Projection to 2x2 Fourier modes: 512 tiny matmuls (out [64c, 8jm] per s,
    accumulated over the 8 hw-chunks) into ONE psum bank -> Xall [c,(s,jm)].
  - QKV complex mixing, batched-head attention (one exp/reduce/recip pass),
    attn@V per head, per-jm transposes, w_out mixing, U_fT via 64 f16
    transposes; stage 7 streams y = U_fT^T @ Bas in 32 chunks of [128,1024]:
    PE matmul (f32 psum) -> DVE/Act half-copies (cast f16) -> DMA out on SP.
"""
import numpy as np

B, S, C, H, W = 8, 64, 64, 32, 32
NH, HD = 8, 8
HW = H * W
NCORES = 8
MODES4 = [(0, 0), (0, 1), (1, 0), (1, 1)]

MAIN_COLS = 3072
SMALL_COLS = 1033


def _constants():
    hh, ww = np.meshgrid(np.arange(H), np.arange(W), indexing="ij")
    phi, psi = 2 * np.pi / H, 2 * np.pi / W
    E8 = np.zeros((HW, 8))
    Bas = np.zeros((8, HW))
    for mi, (kx, ky) in enumerate(MODES4):
        th = phi * kx * hh + psi * ky * ww
        E8[:, 2 * mi] = np.cos(th).ravel()
        E8[:, 2 * mi + 1] = -np.sin(th).ravel()
        mult = 1.0 if ky == 0 else 2.0
        Bas[2 * mi] = mult / HW * np.cos(th).ravel()
        Bas[2 * mi + 1] = -mult / HW * np.sin(th).ravel()
    g = (Bas @ Bas.T).diagonal().copy()      # attention Gram diag (per jm)
    t8d = (Bas @ E8).diagonal().copy()       # coeff->mode map (diagonal)

    e8c = np.zeros((128, 64), np.float32)    # hw-chunk k of E8 at cols [8k,8k+8)
    for k in range(8):
        e8c[:, 8 * k:8 * k + 8] = E8[128 * k:128 * (k + 1)]

    gx, gy = np.meshgrid(np.arange(8), np.arange(8), indexing="ij")
    coords = np.stack([gx.ravel(), gy.ravel()], -1).astype(np.float32)
    rel = coords[:, None, :] - coords[None, :, :]
    rel = np.sign(rel) * np.log2(1.0 + np.abs(rel))          # [64, 64, 2]
    relT = np.ascontiguousarray(rel.reshape(4096, 2).T).astype(np.float32)

    scale = np.float32(1.0 / HW) / np.float32(np.sqrt(HD))
    gcol = np.zeros((64, 1), np.float32)     # rows (jm, d): p = jm*8+d
    for p in range(64):
        gcol[p, 0] = g[p // 8] * scale
    t8pat = np.zeros((64, 512), np.float32)  # over (h, jm, d): col = h*64+jm*8+d
    for col in range(512):
        t8pat[:, col] = t8d[(col // 8) % 8]
    return e8c, Bas.astype(np.float32), relT, gcol, t8pat


def _pack_weights(inputs):
    """mainpack [128, MAIN_COLS] f16, relTpack [2, 4160] f16, bas [8,1024] f16."""
    e8c, bas, relT, gcol, t8pat = _constants()
    wqr = np.asarray(inputs["w_qkv_r"], np.float32).reshape(64, 768)
    wqi = np.asarray(inputs["w_qkv_i"], np.float32).reshape(64, 768)
    wor = np.asarray(inputs["w_out_r"], np.float32).reshape(64, 256)
    woi = np.asarray(inputs["w_out_i"], np.float32).reshape(64, 256)
    cw1 = np.asarray(inputs["cpb_w1"], np.float32)
    cb1 = np.asarray(inputs["cpb_b1"], np.float32).reshape(64, 1)
    cw2 = np.asarray(inputs["cpb_w2"], np.float32)

    main = np.zeros((64, MAIN_COLS), np.float16)
    main[:, 0:768] = wqr
    main[:, 768:1536] = wqi
    main[:, 1536:2304] = -wqi
    main[:, 2304:2560] = wor
    main[:, 2560:2816] = woi
    main[:, 2816:3072] = -woi

    small = np.zeros((64, SMALL_COLS), np.float16)
    small[:, 0:8] = cw2
    small[:, 8:9] = cb1
    small[:, 9:521] = t8pat[:, :512]
    gpat = np.zeros((64, 512), np.float32)
    for p in range(64):
        gpat[p, :] = gcol[p, 0]
    small[:, 521:1033] = gpat

    relTpack = np.zeros((2, 4160), np.float16)
    relTpack[:, 0:4096] = relT
    relTpack[:, 4096:4160] = cw1
    return (e8c.astype(np.float16), main, small, relTpack,
            bas.astype(np.float16))


def _build(debug=False):
    import concourse.bass as bass
    import concourse.mybir as mybir
    import concourse.tile as tile
    from concourse import bacc
    from concourse.masks import make_identity

    f32 = mybir.dt.float32
    f16 = mybir.dt.float16
    Exp = mybir.ActivationFunctionType.Exp
    Relu = mybir.ActivationFunctionType.Relu

    nc = bacc.Bacc("TRN2", target_bir_lowering=False, debug=False,
                   dynamic_dma_scratch_size=49152)
    x_in = nc.dram_tensor("xt", [1024, 4096], f16, kind="ExternalInput")
    e8_in = nc.dram_tensor("e8pack", [128, 64], f16, kind="ExternalInput")
    main_in = nc.dram_tensor("mainpack", [64, MAIN_COLS], f16,
                             kind="ExternalInput")
    small_in = nc.dram_tensor("smallpack", [64, SMALL_COLS], f16,
                              kind="ExternalInput")
    rel_in = nc.dram_tensor("relTpack", [2, 4160], f16, kind="ExternalInput")
    bas_in = nc.dram_tensor("bas", [8, 1024], f16, kind="ExternalInput")
    y_out = nc.dram_tensor("y", [4096, 1024], f16, kind="ExternalOutput")
    dbg_out = {}
    if debug:
        for nm in ("dXsj", "dQ", "dK", "dV", "dbias", "dan", "dU"):
            shp = [8, 4096] if nm == "dU" else [64, 512]
            dbg_out[nm] = nc.dram_tensor(nm, shp, f16, kind="ExternalOutput")

    with tile.TileContext(nc) as tc:
        import contextlib
        ctx = contextlib.ExitStack()
        with ctx:
            singles = ctx.enter_context(tc.tile_pool(name="singles", bufs=1))
            psm = ctx.enter_context(tc.tile_pool(name="psm", bufs=3, space="PSUM"))
            psX = ctx.enter_context(tc.tile_pool(name="psX", bufs=1, space="PSUM"))
            ps7 = ctx.enter_context(tc.tile_pool(name="ps7", bufs=4, space="PSUM"))
            y_pool = ctx.enter_context(tc.tile_pool(name="yp", bufs=8))

            # ---- packed constants: 4 DMAs on scalar; x chunks on sync ----
            relTp = singles.tile([2, 4160], f16, tag="relTp")
            nc.scalar.dma_start(out=relTp[:], in_=rel_in[:])
            e8t = singles.tile([128, 64], f16, tag="e8t")
            nc.scalar.dma_start(out=e8t[:], in_=e8_in[:])
            smallp = singles.tile([64, SMALL_COLS], f16, tag="smallp")
            nc.scalar.dma_start(out=smallp[:], in_=small_in[:])

            xT = []
            for k in range(8):
                t = singles.tile([128, 4096], f16, tag=f"xT{k}")
                nc.sync.dma_start(out=t[:], in_=x_in[128 * k:128 * (k + 1), :])
                xT.append(t)

            mainp = singles.tile([64, MAIN_COLS], f16, tag="mainp")
            nc.sync.dma_start(out=mainp[:], in_=main_in[:])
            bas = singles.tile([8, 1024], f16, tag="bas")
            nc.sync.dma_start(out=bas[:], in_=bas_in[:])

            wqr = mainp[:, 0:768]
            wqi = mainp[:, 768:1536]
            wqin = mainp[:, 1536:2304]
            wor = mainp[:, 2304:2560]
            woi = mainp[:, 2560:2816]
            woin = mainp[:, 2816:3072]
            cw2 = smallp[:, 0:8]
            cb1 = smallp[:, 8:9]
            t8rep = smallp[:, 9:521]
            gpat = smallp[:, 521:1033]
            relT = relTp[:, 0:4096]
            cw1 = relTp[:, 4096:4160]

            ident = singles.tile([64, 64], f16, tag="ident")
            make_identity(nc, ident[:])

            # ---- projection (interleaved with CPB, which needs no x) ----
            # PSUM accumulation groups must be contiguous, so each chunk is a
            # single-shot matmul set into a rotating psum tile, accumulated
            # into SBUF f32 (last chunk writes the f16 result directly).
            Xacc = singles.tile([64, 512], f32, tag="Xacc")
            Xsj = singles.tile([64, 512], f16, tag="Xsj")   # [c, (s, jm)]

            def proj_chunk(k):
                pXk = psX.tile([64, 512], f32, tag="X")
                for s in range(64):
                    nc.tensor.matmul(pXk[:, 8 * s:8 * s + 8],
                                     xT[k][:, 64 * s:64 * (s + 1)],
                                     e8t[:, 8 * k:8 * k + 8],
                                     start=True, stop=True)
                if k == 0:
                    nc.vector.tensor_copy(Xacc[:], pXk[:])
                elif k < 7:
                    nc.vector.tensor_add(Xacc[:], pXk[:], Xacc[:])
                else:
                    nc.vector.tensor_add(Xsj[:], pXk[:], Xacc[:])

            proj_chunk(0)
            proj_chunk(1)
            # CPB layer 1: h_relu [e=64, (i,j)=4096] = relu(cw1^T @ relT + b1)
            h_relu = singles.tile([64, 4096], f16, tag="hrelu")
            for n in range(8):
                pc = psm.tile([64, 512], f32, tag="m")
                nc.tensor.matmul(pc[:], cw1[:], relT[:, 512 * n:512 * (n + 1)],
                                 start=True, stop=True)
                nc.scalar.activation(h_relu[:, 512 * n:512 * (n + 1)], pc[:],
                                     Relu, bias=cb1[:])
            proj_chunk(2)
            proj_chunk(3)
            # CPB layer 2: bias [i, (j, h)]: 64 matmuls, one copy out
            h3 = h_relu.rearrange("e (i j) -> e i j", j=64)
            pb = psm.tile([64, 512], f32, tag="m")
            for j in range(64):
                nc.tensor.matmul(pb[:, 8 * j:8 * j + 8], h3[:, :, j], cw2[:],
                                 start=True, stop=True)
            bias_jh = singles.tile([64, 512], f32, tag="bias_jh")  # [i, (j, h)]
            nc.vector.tensor_copy(bias_jh[:], pb[:])
            for k in range(4, 8):
                proj_chunk(k)

            xv = Xsj.rearrange("c (s j) -> c j s", j=8)

            # ---- QKV complex mixing -> Q/K/V [s, (h, jm, d)] ----
            wq3 = wqr.rearrange("c (o m) -> c o m", m=4)
            wi3 = wqi.rearrange("c (o m) -> c o m", m=4)
            win3 = wqin.rearrange("c (o m) -> c o m", m=4)
            Q_sb = singles.tile([64, 512], f16, tag="Qsb")
            K_sb = singles.tile([64, 512], f16, tag="Ksb")
            V_sb = singles.tile([64, 512], f16, tag="Vsb")
            for qi, (sb, o0) in enumerate(((Q_sb, 0), (K_sb, 64), (V_sb, 128))):
                pd = psm.tile([64, 512], f32, tag="m")
                for m in range(4):
                    lR = xv[:, 2 * m, :]
                    lI = xv[:, 2 * m + 1, :]
                    wR = wq3[:, o0:o0 + 64, m]
                    wI = wi3[:, o0:o0 + 64, m]
                    wIn = win3[:, o0:o0 + 64, m]
                    blk = pd[:, 64 * (2 * m):64 * (2 * m) + 64]
                    nc.tensor.matmul(blk, lR, wR, start=True, stop=False)
                    nc.tensor.matmul(blk, lI, wIn, start=False, stop=True)
                    blk = pd[:, 64 * (2 * m + 1):64 * (2 * m + 1) + 64]
                    nc.tensor.matmul(blk, lR, wI, start=True, stop=False)
                    nc.tensor.matmul(blk, lI, wR, start=False, stop=True)
                pdv = pd.rearrange("s (j h d) -> s j h d", h=8, d=8)
                sbv = sb.rearrange("s (h j d) -> s j h d", j=8, d=8)
                if qi == 0:
                    nc.vector.tensor_copy(sbv[:], pdv[:])
                elif qi == 1:
                    nc.scalar.copy(sbv[:], pdv[:])
                else:
                    # V scaled by t8 diag (fold of the coeff->mode map);
                    # t8rep pattern is laid out for the (h, jm, d) output order
                    nc.vector.tensor_mul(
                        sbv[:], pdv[:],
                        t8rep.rearrange("s (h j d) -> s j h d", j=8, d=8)[:])

            # ---- attention ----
            QT = singles.tile([64, 512], f16, tag="QT")
            KT = singles.tile([64, 512], f16, tag="KT")
            pqt = psm.tile([64, 512], f16, tag="m")
            for h in range(8):
                nc.tensor.transpose(pqt[:, 64 * h:64 * (h + 1)],
                                    Q_sb[:, 64 * h:64 * (h + 1)], ident[:])
            nc.vector.tensor_mul(QT[:], pqt[:], gpat[:])
            pkt = psm.tile([64, 512], f16, tag="m")
            for h in range(8):
                nc.tensor.transpose(pkt[:, 64 * h:64 * (h + 1)],
                                    K_sb[:, 64 * h:64 * (h + 1)], ident[:])
            nc.scalar.copy(KT[:], pkt[:])

            # scores per head into one psum bank [i, (h, j)]
            pS = psm.tile([64, 512], f32, tag="m")
            for h in range(8):
                nc.tensor.matmul(pS[:, 64 * h:64 * (h + 1)],
                                 QT[:, 64 * h:64 * (h + 1)],
                                 KT[:, 64 * h:64 * (h + 1)],
                                 start=True, stop=True)
            # add bias and exp, pipelined in head-halves across DVE/Act
            sc_sb = singles.tile([64, 512], f32, tag="sc")
            ex = singles.tile([64, 512], f32, tag="ex")
            se = singles.tile([64, 8], f32, tag="se")
            ri = singles.tile([64, 8], f32, tag="ri")
            bview = bias_jh.rearrange("i (j h) -> i h j", h=8)
            sview = sc_sb.rearrange("i (h j) -> i h j", j=64)
            pview = pS.rearrange("i (h j) -> i h j", j=64)
            eview = ex.rearrange("i (h j) -> i h j", j=64)
            for hh in range(2):
                hs = slice(4 * hh, 4 * (hh + 1))
                nc.vector.tensor_add(sview[:, hs], pview[:, hs], bview[:, hs])
                nc.scalar.activation(ex[:, 256 * hh:256 * (hh + 1)],
                                     sc_sb[:, 256 * hh:256 * (hh + 1)], Exp)
                nc.vector.reduce_sum(se[:, hs], eview[:, hs],
                                     axis=mybir.AxisListType.X)
            nc.vector.reciprocal(ri[:], se[:])
            # normalized attention (f16) [i, (h, j)]
            an = singles.tile([64, 512], f16, tag="an")
            nc.vector.tensor_mul(
                an.rearrange("i (h j) -> i h j", j=64)[:],
                eview[:],
                ri[:, :, None].to_broadcast([64, 8, 64]))
            # transpose attention per head -> amT [j, (h, i)]
            pat = psm.tile([64, 512], f16, tag="m")
            for h in range(8):
                nc.tensor.transpose(pat[:, 64 * h:64 * (h + 1)],
                                    an[:, 64 * h:64 * (h + 1)], ident[:])
            amT = singles.tile([64, 512], f16, tag="amT")
            nc.vector.tensor_copy(amT[:], pat[:])

            # ---- attn @ V -> O [i, (h, jm, d)], then per-jm transposes ----
            pO = psm.tile([64, 512], f32, tag="m")
            for h in range(8):
                nc.tensor.matmul(pO[:, 64 * h:64 * (h + 1)],
                                 amT[:, 64 * h:64 * (h + 1)],
                                 V_sb[:, 64 * h:64 * (h + 1)],
                                 start=True, stop=True)
            O_sb = singles.tile([64, 512], f16, tag="Osb")  # [i, (jm, h, d)]
            nc.vector.tensor_copy(
                O_sb.rearrange("i (j h d) -> i j h d", h=8, d=8)[:],
                pO.rearrange("i (h j d) -> i j h d", j=8, d=8)[:])
            # transpose [s, (h,d)] slice per jm -> [ (h,d), s ]; XOR | XOI
            pXO = psm.tile([64, 512], f16, tag="m")
            for m in range(4):
                nc.tensor.transpose(pXO[:, 64 * m:64 * (m + 1)],
                                    O_sb[:, 128 * m:128 * m + 64], ident[:])
                nc.tensor.transpose(pXO[:, 256 + 64 * m:256 + 64 * (m + 1)],
                                    O_sb[:, 128 * m + 64:128 * (m + 1)], ident[:])
            XO = singles.tile([64, 512], f16, tag="XO")
            nc.vector.tensor_copy(XO[:], pXO[:])
            XOR = XO[:, 0:256].rearrange("c (m s) -> c m s", s=64)
            XOI = XO[:, 256:512].rearrange("c (m s) -> c m s", s=64)

            # ---- w_out complex mixing -> F [c_out, (jm, s)] ----
            wo3 = wor.rearrange("c (o m) -> c o m", m=4)
            woi3 = woi.rearrange("c (o m) -> c o m", m=4)
            woin3 = woin.rearrange("c (o m) -> c o m", m=4)
            pf = psm.tile([64, 512], f32, tag="m")
            for m in range(4):
                rR = XOR[:, m, :]
                rI = XOI[:, m, :]
                blk = pf[:, 64 * (2 * m):64 * (2 * m) + 64]
                nc.tensor.matmul(blk, wo3[:, :, m], rR, start=True, stop=False)
                nc.tensor.matmul(blk, woin3[:, :, m], rI, start=False, stop=True)
                blk = pf[:, 64 * (2 * m + 1):64 * (2 * m + 1) + 64]
                nc.tensor.matmul(blk, woi3[:, :, m], rR, start=True, stop=False)
                nc.tensor.matmul(blk, wo3[:, :, m], rI, start=False, stop=True)
            F_sb = singles.tile([64, 512], f16, tag="Fsb")
            nc.vector.tensor_copy(F_sb[:], pf[:])

            # ---- U_fT [8, (s, c)] via 64 cheap f16 transposes ----
            f3 = F_sb.rearrange("c (j s) -> c j s", s=64)
            U_fT = []
            for g in range(8):
                uft_g = singles.tile([8, 512], f16, tag=f"UfT{g}")
                U_fT.append(uft_g)
            for g8 in range(8):
                pu = psm.tile([8, 512], f16, tag="m")
                for u in range(8):
                    s = 8 * g8 + u
                    nc.tensor.transpose(pu[:, 64 * u:64 * (u + 1)],
                                        f3[:, :, s], ident[:])
                if g8 % 2 == 0:
                    nc.vector.tensor_copy(U_fT[g8][:], pu[:])
                else:
                    nc.scalar.copy(U_fT[g8][:], pu[:])

            if debug:
                nc.sync.dma_start(out=dbg_out["dXsj"][:], in_=Xsj[:])
                nc.sync.dma_start(out=dbg_out["dQ"][:], in_=Q_sb[:])
                nc.sync.dma_start(out=dbg_out["dK"][:], in_=K_sb[:])
                nc.sync.dma_start(out=dbg_out["dV"][:], in_=V_sb[:])
                nc.scalar.dma_start(out=dbg_out["dan"][:], in_=an[:])
                dbias16 = singles.tile([64, 512], f16, tag="dbias16")
                nc.vector.tensor_copy(dbias16[:], bias_jh[:])
                nc.scalar.dma_start(out=dbg_out["dbias"][:], in_=dbias16[:])

            # ---- stage 7: y = U_fT^T @ Bas, streamed in 32 chunks ----
            cp_engines = [nc.vector, nc.scalar]
            for t in range(32):
                lh = U_fT[t // 4][:, 128 * (t % 4):128 * (t % 4 + 1)]
                py0 = ps7.tile([128, 512], f32, tag="y")
                py1 = ps7.tile([128, 512], f32, tag="y")
                nc.tensor.matmul(py0[:], lh, bas[:, :512], start=True, stop=True)
                nc.tensor.matmul(py1[:], lh, bas[:, 512:], start=True, stop=True)
                y_sb = y_pool.tile([128, 1024], f16, tag="y_sb")
                for half, py in ((0, py0), (1, py1)):
                    ce = cp_engines[(t + half) % 2]
                    dstv = y_sb[:, 512 * half:512 * (half + 1)]
                    if ce is nc.scalar:
                        ce.copy(dstv, py[:])
                    else:
                        ce.tensor_copy(dstv, py[:])
                (nc.scalar if t % 3 == 2 else nc.sync).dma_start(
                    out=y_out[128 * t:128 * (t + 1), :], in_=y_sb[:])
    nc.finalize()
    return nc


_NC_CACHE = {}


def kernel(**inputs) -> np.ndarray:
    from concourse.bass_utils import run_bass_kernel_spmd

    seq = np.asarray(inputs["seq"], dtype=np.float32)
    assert seq.shape == (B, S, C, H, W)

    if "nc" not in _NC_CACHE:
        _NC_CACHE["nc"] = _build()
    nc = _NC_CACHE["nc"]

    e8pk, main, small, relTpack, bas16 = _pack_weights(inputs)
    common = {"e8pack": e8pk, "mainpack": main, "smallpack": small,
              "relTpack": relTpack, "bas": bas16}
    seq16 = seq.reshape(B, 4096, 1024).astype(np.float16)
    in_maps = []
    for b in range(NCORES):
        m = dict(common)
        m["xt"] = np.ascontiguousarray(seq16[b].T)
        in_maps.append(m)

    res = run_bass_kernel_spmd(nc, in_maps, list(range(NCORES)))
    out = np.stack([res.results[b]["y"].astype(np.float32).reshape(S, C, H, W)
                    for b in range(NCORES)])
    return out


# revision 30
# speedup vs baseline: 1.9093x; 1.0017x over previous
"""Trainium2 Bass kernel for nn_FuncSelfAttention (spectral self-attention).

v3 design (cost-model driven):
  - Host casts seq to f16 AND pre-transposes it: x^T [1024 hw, 4096 (s,c)]
    per core (data-parallel over batch, 1 batch element per core). Device
    does 8 plain 1MB DMA loads (no transpose premium, no queue clog).
  - All weights/constants packed host-side into 3 DMAs (ring depth per
    HWDGE queue is 2, so many small loads would serialize at ~1 per big
    transfer otherwise).
  - Projection to 2x2 Fourier modes: 512 tiny matmuls (out [64c, 8jm] per s,
    accumulated over the 8 hw-chunks) into ONE psum bank -> Xall [c,(s,jm)].
  - QKV complex mixing, batched-head attention (one exp/reduce/recip pass),
    attn@V per head, per-jm transposes, w_out mixing, U_fT via 64 f16
    transposes; stage 7 streams y = U_fT^T @ Bas in 32 chunks of [128,1024]:
    PE matmul (f32 psum) -> DVE/Act half-copies (cast f16) -> DMA out on SP.
"""
import numpy as np

B, S, C, H, W = 8, 64, 64, 32, 32
NH, HD = 8, 8
HW = H * W
NCORES = 8
MODES4 = [(0, 0), (0, 1), (1, 0), (1, 1)]

MAIN_COLS = 3072
SMALL_COLS = 1033


def _constants():
    hh, ww = np.meshgrid(np.arange(H), np.arange(W), indexing="ij")
    phi, psi = 2 * np.pi / H, 2 * np.pi / W
    E8 = np.zeros((HW, 8))
    Bas = np.zeros((8, HW))
    for mi, (kx, ky) in enumerate(MODES4):
        th = phi * kx * hh + psi * ky * ww
        E8[:, 2 * mi] = np.cos(th).ravel()
        E8[:, 2 * mi + 1] = -np.sin(th).ravel()
        mult = 1.0 if ky == 0 else 2.0
        Bas[2 * mi] = mult / HW * np.cos(th).ravel()
        Bas[2 * mi + 1] = -mult / HW * np.sin(th).ravel()
    g = (Bas @ Bas.T).diagonal().copy()      # attention Gram diag (per jm)
    t8d = (Bas @ E8).diagonal().copy()       # coeff->mode map (diagonal)

    e8c = np.zeros((128, 64), np.float32)    # hw-chunk k of E8 at cols [8k,8k+8)
    for k in range(8):
        e8c[:, 8 * k:8 * k + 8] = E8[128 * k:128 * (k + 1)]

    gx, gy = np.meshgrid(np.arange(8), np.arange(8), indexing="ij")
    coords = np.stack([gx.ravel(), gy.ravel()], -1).astype(np.float32)
    rel = coords[:, None, :] - coords[None, :, :]
    rel = np.sign(rel) * np.log2(1.0 + np.abs(rel))          # [64, 64, 2]
    relT = np.ascontiguousarray(rel.reshape(4096, 2).T).astype(np.float32)

    scale = np.float32(1.0 / HW) / np.float32(np.sqrt(HD))
    gcol = np.zeros((64, 1), np.float32)     # rows (jm, d): p = jm*8+d
    for p in range(64):
        gcol[p, 0] = g[p // 8] * scale
    t8pat = np.zeros((64, 512), np.float32)  # over (h, jm, d): col = h*64+jm*8+d
    for col in range(512):
        t8pat[:, col] = t8d[(col // 8) % 8]
    return e8c, Bas.astype(np.float32), relT, gcol, t8pat


def _pack_weights(inputs):
    """mainpack [128, MAIN_COLS] f16, relTpack [2, 4160] f16, bas [8,1024] f16."""
    e8c, bas, relT, gcol, t8pat = _constants()
    wqr = np.asarray(inputs["w_qkv_r"], np.float32).reshape(64, 768)
    wqi = np.asarray(inputs["w_qkv_i"], np.float32).reshape(64, 768)
    wor = np.asarray(inputs["w_out_r"], np.float32).reshape(64, 256)
    woi = np.asarray(inputs["w_out_i"], np.float32).reshape(64, 256)
    cw1 = np.asarray(inputs["cpb_w1"], np.float32)
    cb1 = np.asarray(inputs["cpb_b1"], np.float32).reshape(64, 1)
    cw2 = np.asarray(inputs["cpb_w2"], np.float32)

    main = np.zeros((64, MAIN_COLS), np.float16)
    main[:, 0:768] = wqr
    main[:, 768:1536] = wqi
    main[:, 1536:2304] = -wqi
    main[:, 2304:2560] = wor
    main[:, 2560:2816] = woi
    main[:, 2816:3072] = -woi

    small = np.zeros((64, SMALL_COLS), np.float16)
    small[:, 0:8] = cw2
    small[:, 8:9] = cb1
    small[:, 9:521] = t8pat[:, :512]
    gpat = np.zeros((64, 512), np.float32)
    for p in range(64):
        gpat[p, :] = gcol[p, 0]
    small[:, 521:1033] = gpat

    relTpack = np.zeros((2, 4160), np.float16)
    relTpack[:, 0:4096] = relT
    relTpack[:, 4096:4160] = cw1
    return (e8c.astype(np.float16), main, small, relTpack,
            bas.astype(np.float16))


def _build(debug=False):
    import concourse.bass as bass
    import concourse.mybir as mybir
    import concourse.tile as tile
    from concourse import bacc
    from concourse.masks import make_identity

    f32 = mybir.dt.float32
    f16 = mybir.dt.float16
    Exp = mybir.ActivationFunctionType.Exp
    Relu = mybir.ActivationFunctionType.Relu

    nc = bacc.Bacc("TRN2", target_bir_lowering=False, debug=False,
                   dynamic_dma_scratch_size=49152)
    x_in = nc.dram_tensor("xt", [1024, 4096], f16, kind="ExternalInput")
    e8_in = nc.dram_tensor("e8pack", [128, 64], f16, kind="ExternalInput")
    main_in = nc.dram_tensor("mainpack", [64, MAIN_COLS], f16,
                             kind="ExternalInput")
    small_in = nc.dram_tensor("smallpack", [64, SMALL_COLS], f16,
                              kind="ExternalInput")
    rel_in = nc.dram_tensor("relTpack", [2, 4160], f16, kind="ExternalInput")
    bas_in = nc.dram_tensor("bas", [8, 1024], f16, kind="ExternalInput")
    y_out = nc.dram_tensor("y", [4096, 1024], f16, kind="ExternalOutput")
    dbg_out = {}
    if debug:
        for nm in ("dXsj", "dQ", "dK", "dV", "dbias", "dan", "dU"):
            shp = [8, 4096] if nm == "dU" else [64, 512]
            dbg_out[nm] = nc.dram_tensor(nm, shp, f16, kind="ExternalOutput")

    with tile.TileContext(nc) as tc:
        import contextlib
        ctx = contextlib.ExitStack()
        with ctx:
            singles = ctx.enter_context(tc.tile_pool(name="singles", bufs=1))
            psm = ctx.enter_context(tc.tile_pool(name="psm", bufs=3, space="PSUM"))
            psX = ctx.enter_context(tc.tile_pool(name="psX", bufs=1, space="PSUM"))
            ps7 = ctx.enter_context(tc.tile_pool(name="ps7", bufs=4, space="PSUM"))
            y_pool = ctx.enter_context(tc.tile_pool(name="yp", bufs=8))

            # ---- packed constants: 4 DMAs on scalar; x chunks on sync ----
            relTp = singles.tile([2, 4160], f16, tag="relTp")
            nc.scalar.dma_start(out=relTp[:], in_=rel_in[:])
            e8t = singles.tile([128, 64], f16, tag="e8t")
            nc.scalar.dma_start(out=e8t[:], in_=e8_in[:])
            smallp = singles.tile([64, SMALL_COLS], f16, tag="smallp")
            nc.scalar.dma_start(out=smallp[:], in_=small_in[:])

            xT = []
            for k in range(8):
                t = singles.tile([128, 4096], f16, tag=f"xT{k}")
                nc.sync.dma_start(out=t[:], in_=x_in[128 * k:128 * (k + 1), :])
                xT.append(t)

            mainp = singles.tile([64, MAIN_COLS], f16, tag="mainp")
            nc.sync.dma_start(out=mainp[:], in_=main_in[:])
            bas = singles.tile([8, 1024], f16, tag="bas")
            nc.sync.dma_start(out=bas[:], in_=bas_in[:])

            wqr = mainp[:, 0:768]
            wqi = mainp[:, 768:1536]
            wqin = mainp[:, 1536:2304]
            wor = mainp[:, 2304:2560]
            woi = mainp[:, 2560:2816]
            woin = mainp[:, 2816:3072]
            cw2 = smallp[:, 0:8]
            cb1 = smallp[:, 8:9]
            t8rep = smallp[:, 9:521]
            gpat = smallp[:, 521:1033]
            relT = relTp[:, 0:4096]
            cw1 = relTp[:, 4096:4160]

            ident = singles.tile([64, 64], f16, tag="ident")
            make_identity(nc, ident[:])

            # ---- projection (interleaved with CPB, which needs no x) ----
            # PSUM accumulation groups must be contiguous, so each chunk is a
            # single-shot matmul set into a rotating psum tile, accumulated
            # into SBUF f32 (last chunk writes the f16 result directly).
            Xacc = singles.tile([64, 512], f32, tag="Xacc")
            Xsj = singles.tile([64, 512], f16, tag="Xsj")   # [c, (s, jm)]

            def proj_chunk(k):
                pXk = psX.tile([64, 512], f32, tag="X")
                for s in range(64):
                    nc.tensor.matmul(pXk[:, 8 * s:8 * s + 8],
                                     xT[k][:, 64 * s:64 * (s + 1)],
                                     e8t[:, 8 * k:8 * k + 8],
                                     start=True, stop=True)
                if k == 0:
                    nc.vector.tensor_copy(Xacc[:], pXk[:])
                elif k < 7:
                    nc.vector.tensor_add(Xacc[:], pXk[:], Xacc[:])
                else:
                    nc.vector.tensor_add(Xsj[:], pXk[:], Xacc[:])

            proj_chunk(0)
            proj_chunk(1)
            # CPB layer 1: h_relu [e=64, (i,j)=4096] = relu(cw1^T @ relT + b1)
            h_relu = singles.tile([64, 4096], f16, tag="hrelu")
            for n in range(8):
                pc = psm.tile([64, 512], f32, tag="m")
                nc.tensor.matmul(pc[:], cw1[:], relT[:, 512 * n:512 * (n + 1)],
                                 start=True, stop=True)
                nc.scalar.activation(h_relu[:, 512 * n:512 * (n + 1)], pc[:],
                                     Relu, bias=cb1[:])
            proj_chunk(2)
            proj_chunk(3)
            # CPB layer 2: bias [i, (j, h)]: 64 matmuls, one copy out
            h3 = h_relu.rearrange("e (i j) -> e i j", j=64)
            pb = psm.tile([64, 512], f32, tag="m")
            for j in range(64):
                nc.tensor.matmul(pb[:, 8 * j:8 * j + 8], h3[:, :, j], cw2[:],
                                 start=True, stop=True)
            bias_jh = singles.tile([64, 512], f32, tag="bias_jh")  # [i, (j, h)]
            nc.vector.tensor_copy(bias_jh[:], pb[:])
            for k in range(4, 8):
                proj_chunk(k)

            xv = Xsj.rearrange("c (s j) -> c j s", j=8)

            # ---- QKV complex mixing -> Q/K/V [s, (h, jm, d)] ----
            wq3 = wqr.rearrange("c (o m) -> c o m", m=4)
            wi3 = wqi.rearrange("c (o m) -> c o m", m=4)
            win3 = wqin.rearrange("c (o m) -> c o m", m=4)
            Q_sb = singles.tile([64, 512], f16, tag="Qsb")
            K_sb = singles.tile([64, 512], f16, tag="Ksb")
            V_sb = singles.tile([64, 512], f16, tag="Vsb")
            for qi, (sb, o0) in enumerate(((Q_sb, 0), (K_sb, 64), (V_sb, 128))):
                pd = psm.tile([64, 512], f32, tag="m")
                for m in range(4):
                    lR = xv[:, 2 * m, :]
                    lI = xv[:, 2 * m + 1, :]
                    wR = wq3[:, o0:o0 + 64, m]
                    wI = wi3[:, o0:o0 + 64, m]
                    wIn = win3[:, o0:o0 + 64, m]
                    blk = pd[:, 64 * (2 * m):64 * (2 * m) + 64]
                    nc.tensor.matmul(blk, lR, wR, start=True, stop=False)
                    nc.tensor.matmul(blk, lI, wIn, start=False, stop=True)
                    blk = pd[:, 64 * (2 * m + 1):64 * (2 * m + 1) + 64]
                    nc.tensor.matmul(blk, lR, wI, start=True, stop=False)
                    nc.tensor.matmul(blk, lI, wR, start=False, stop=True)
                pdv = pd.rearrange("s (j h d) -> s j h d", h=8, d=8)
                sbv = sb.rearrange("s (h j d) -> s j h d", j=8, d=8)
                if qi == 0:
                    nc.vector.tensor_copy(sbv[:], pdv[:])
                elif qi == 1:
                    nc.scalar.copy(sbv[:], pdv[:])
                else:
                    # V scaled by t8 diag (fold of the coeff->mode map);
                    # t8rep pattern is laid out for the (h, jm, d) output order
                    nc.vector.tensor_mul(
                        sbv[:], pdv[:],
                        t8rep.rearrange("s (h j d) -> s j h d", j=8, d=8)[:])

            # ---- attention ----
            QT = singles.tile([64, 512], f16, tag="QT")
            KT = singles.tile([64, 512], f16, tag="KT")
            pqt = psm.tile([64, 512], f16, tag="m")
            for h in range(8):
                nc.tensor.transpose(pqt[:, 64 * h:64 * (h + 1)],
                                    Q_sb[:, 64 * h:64 * (h + 1)], ident[:])
            nc.vector.tensor_mul(QT[:], pqt[:], gpat[:])
            pkt = psm.tile([64, 512], f16, tag="m")
            for h in range(8):
                nc.tensor.transpose(pkt[:, 64 * h:64 * (h + 1)],
                                    K_sb[:, 64 * h:64 * (h + 1)], ident[:])
            nc.scalar.copy(KT[:], pkt[:])

            # scores per head into one psum bank [i, (h, j)]
            pS = psm.tile([64, 512], f32, tag="m")
            for h in range(8):
                nc.tensor.matmul(pS[:, 64 * h:64 * (h + 1)],
                                 QT[:, 64 * h:64 * (h + 1)],
                                 KT[:, 64 * h:64 * (h + 1)],
                                 start=True, stop=True)
            # add bias and exp, pipelined in head-halves across DVE/Act
            sc_sb = singles.tile([64, 512], f32, tag="sc")
            ex = singles.tile([64, 512], f32, tag="ex")
            se = singles.tile([64, 8], f32, tag="se")
            ri = singles.tile([64, 8], f32, tag="ri")
            bview = bias_jh.rearrange("i (j h) -> i h j", h=8)
            sview = sc_sb.rearrange("i (h j) -> i h j", j=64)
            pview = pS.rearrange("i (h j) -> i h j", j=64)
            eview = ex.rearrange("i (h j) -> i h j", j=64)
            for hh in range(2):
                hs = slice(4 * hh, 4 * (hh + 1))
                nc.vector.tensor_add(sview[:, hs], pview[:, hs], bview[:, hs])
                nc.scalar.activation(ex[:, 256 * hh:256 * (hh + 1)],
                                     sc_sb[:, 256 * hh:256 * (hh + 1)], Exp)
                nc.vector.reduce_sum(se[:, hs], eview[:, hs],
                                     axis=mybir.AxisListType.X)
            nc.vector.reciprocal(ri[:], se[:])
            # normalized attention (f16) [i, (h, j)]
            an = singles.tile([64, 512], f16, tag="an")
            nc.vector.tensor_mul(
                an.rearrange("i (h j) -> i h j", j=64)[:],
                eview[:],
                ri[:, :, None].to_broadcast([64, 8, 64]))
            # transpose attention per head -> amT [j, (h, i)]
            pat = psm.tile([64, 512], f16, tag="m")
            for h in range(8):
                nc.tensor.transpose(pat[:, 64 * h:64 * (h + 1)],
                                    an[:, 64 * h:64 * (h + 1)], ident[:])
            amT = singles.tile([64, 512], f16, tag="amT")
            nc.vector.tensor_copy(amT[:], pat[:])

            # ---- attn @ V -> O [i, (h, jm, d)], then per-jm transposes ----
            pO = psm.tile([64, 512], f32, tag="m")
            for h in range(8):
                nc.tensor.matmul(pO[:, 64 * h:64 * (h + 1)],
                                 amT[:, 64 * h:64 * (h + 1)],
                                 V_sb[:, 64 * h:64 * (h + 1)],
                                 start=True, stop=True)
            O_sb = singles.tile([64, 512], f16, tag="Osb")  # [i, (jm, h, d)]
            nc.vector.tensor_copy(
                O_sb.rearrange("i (j h d) -> i j h d", h=8, d=8)[:],
                pO.rearrange("i (h j d) -> i j h d", j=8, d=8)[:])
            # transpose [s, (h,d)] slice per jm -> [ (h,d), s ]; XOR | XOI
            pXO = psm.tile([64, 512], f16, tag="m")
            for m in range(4):
                nc.tensor.transpose(pXO[:, 64 * m:64 * (m + 1)],
                                    O_sb[:, 128 * m:128 * m + 64], ident[:])
                nc.tensor.transpose(pXO[:, 256 + 64 * m:256 + 64 * (m + 1)],
                                    O_sb[:, 128 * m + 64:128 * (m + 1)], ident[:])
            XO = singles.tile([64, 512], f16, tag="XO")
            nc.vector.tensor_copy(XO[:], pXO[:])
            XOR = XO[:, 0:256].rearrange("c (m s) -> c m s", s=64)
            XOI = XO[:, 256:512].rearrange("c (m s) -> c m s", s=64)

            # ---- w_out complex mixing -> F [c_out, (jm, s)] ----
            wo3 = wor.rearrange("c (o m) -> c o m", m=4)
            woi3 = woi.rearrange("c (o m) -> c o m", m=4)
            woin3 = woin.rearrange("c (o m) -> c o m", m=4)
            pf = psm.tile([64, 512], f32, tag="m")
            for m in range(4):
                rR = XOR[:, m, :]
                rI = XOI[:, m, :]
                blk = pf[:, 64 * (2 * m):64 * (2 * m) + 64]
                nc.tensor.matmul(blk, wo3[:, :, m], rR, start=True, stop=False)
                nc.tensor.matmul(blk, woin3[:, :, m], rI, start=False, stop=True)
                blk = pf[:, 64 * (2 * m + 1):64 * (2 * m + 1) + 64]
                nc.tensor.matmul(blk, woi3[:, :, m], rR, start=True, stop=False)
                nc.tensor.matmul(blk, wo3[:, :, m], rI, start=False, stop=True)
            F_sb = singles.tile([64, 512], f16, tag="Fsb")
            nc.vector.tensor_copy(F_sb[:], pf[:])

            # ---- U_fT [8, (s, c)] via 64 cheap f16 transposes ----
            f3 = F_sb.rearrange("c (j s) -> c j s", s=64)
            U_fT = []
            for g in range(8):
                uft_g = singles.tile([8, 512], f16, tag=f"UfT{g}")
                U_fT.append(uft_g)
            for g8 in range(8):
                pu = psm.tile([8, 512], f16, tag="m")
                for u in range(8):
                    s = 8 * g8 + u
                    nc.tensor.transpose(pu[:, 64 * u:64 * (u + 1)],
                                        f3[:, :, s], ident[:])
                if g8 % 2 == 0:
                    nc.vector.tensor_copy(U_fT[g8][:], pu[:])
                else:
                    nc.scalar.copy(U_fT[g8][:], pu[:])

            if debug:
                nc.sync.dma_start(out=dbg_out["dXsj"][:], in_=Xsj[:])
                nc.sync.dma_start(out=dbg_out["dQ"][:], in_=Q_sb[:])
                nc.sync.dma_start(out=dbg_out["dK"][:], in_=K_sb[:])
                nc.sync.dma_start(out=dbg_out["dV"][:], in_=V_sb[:])
                nc.scalar.dma_start(out=dbg_out["dan"][:], in_=an[:])
                dbias16 = singles.tile([64, 512], f16, tag="dbias16")
                nc.vector.tensor_copy(dbias16[:], bias_jh[:])
                nc.scalar.dma_start(out=dbg_out["dbias"][:], in_=dbias16[:])

            # ---- stage 7: y = U_fT^T @ Bas, streamed in 32 chunks ----
            cp_engines = [nc.vector, nc.scalar]
            for t in range(32):
                lh = U_fT[t // 4][:, 128 * (t % 4):128 * (t % 4 + 1)]
                py0 = ps7.tile([128, 512], f32, tag="y")
                py1 = ps7.tile([128, 512], f32, tag="y")
                nc.tensor.matmul(py0[:], lh, bas[:, :512], start=True, stop=True)
                nc.tensor.matmul(py1[:], lh, bas[:, 512:], start=True, stop=True)
                y_sb = y_pool.tile([128, 1024], f16, tag="y_sb")
                for half, py in ((0, py0), (1, py1)):
                    ce = cp_engines[(t + half) % 2]
                    dstv = y_sb[:, 512 * half:512 * (half + 1)]
                    if ce is nc.scalar:
                        ce.copy(dstv, py[:])
                    else:
                        ce.tensor_copy(dstv, py[:])
                (nc.scalar if t % 4 == 3 else nc.sync).dma_start(
                    out=y_out[128 * t:128 * (t + 1), :], in_=y_sb[:])
    nc.finalize()
    return nc


_NC_CACHE = {}


def kernel(**inputs) -> np.ndarray:
    from concourse.bass_utils import run_bass_kernel_spmd

    seq = np.asarray(inputs["seq"], dtype=np.float32)
    assert seq.shape == (B, S, C, H, W)

    if "nc" not in _NC_CACHE:
        _NC_CACHE["nc"] = _build()
    nc = _NC_CACHE["nc"]

    e8pk, main, small, relTpack, bas16 = _pack_weights(inputs)
    common = {"e8pack": e8pk, "mainpack": main, "smallpack": small,
              "relTpack": relTpack, "bas": bas16}
    seq16 = seq.reshape(B, 4096, 1024).astype(np.float16)
    in_maps = []
    for b in range(NCORES):
        m = dict(common)
        m["xt"] = np.ascontiguousarray(seq16[b].T)
        in_maps.append(m)

    res = run_bass_kernel_spmd(nc, in_maps, list(range(NCORES)))
    out = np.stack([res.results[b]["y"].astype(np.float32).reshape(S, C, H, W)
                    for b in range(NCORES)])
    return out
